# revision 1
# baseline (speedup 1.0000x reference)
"""Causal MHA with RoPE on 8 Trainium2 NeuronCores.

Sharding: tensor-parallel over heads. Core c owns heads {2c, 2c+1} (a 128-wide
slice of the model dim). Each core computes Q/K/V projections for its heads,
full causal attention, and a partial o_proj; the host sums the 8 partial
outputs (the "all-reduce").

v2: software-pipelined across batches; fine-grained causal trimming on the
diagonal; RoPE via host-permuted sin table (u = ps*sinp, swap matmul, add);
softmax denominators normalized with direct DVE reciprocal on the replicated
ones-rows of the PV output (no transpose/broadcast matmuls).

Device layouts (per core):
  x.T   [128 i-part, 8 i-tile, t]  bf16, via DMA xbar transpose of bf16 x
  qk_sb [128 hd, 2(q/k), t] bf16;  hd = [head A (ev 0:32, od 32:64), head B]
  scores S.T [k, q] per 128-key tile; P = exp(S.T/8) bf16 in SBUF
  PV: v_sb k-tiles [VA(64) | ones(64) | VB(64)]; ones rows give denominators
  o_proj: ot [128 hd, t] bf16 (stationary) x Wo.T [128 hd, 1024] -> y bf16
"""
import sys
sys.path.insert(0, '/opt/trn_rl_repo')

import numpy as np
import ml_dtypes

import concourse.bass as bass
from concourse import bacc
import concourse.mybir as mybir
import concourse.tile as tile
from concourse.bass_utils import run_bass_kernel_spmd

BFNP = ml_dtypes.bfloat16
F32 = mybir.dt.float32
BF16 = mybir.dt.bfloat16
AF = mybir.ActivationFunctionType

B, S, D = 4, 2048, 1024
NCORES = 8
BS = B * S
ROPE_THETA = 10000.0

TRACE = False
LAST_RESULTS = None
PE_LABELS = []
PE_LABEL_BY_NAME = {}


def build_nc(nb=B):
    global PE_LABELS
    PE_LABELS = []
    nc = bacc.Bacc()

    _mm = nc.tensor.matmul
    def _mm_tagged(*a, _label="?", **k):
        PE_LABELS.append(_label)
        r = _mm(*a, **k)
        PE_LABEL_BY_NAME[r.ins.name] = _label
        return r
    nc.tensor.matmul = _mm_tagged
    xbt = nc.dram_tensor("xbt", [128, 8, BS], BF16, kind="ExternalInput")
    wqt = nc.dram_tensor("wqt", [128, 8, 128], BF16, kind="ExternalInput")
    wkt = nc.dram_tensor("wkt", [128, 8, 128], BF16, kind="ExternalInput")
    wvt = nc.dram_tensor("wvt", [128, 8, 128], BF16, kind="ExternalInput")
    wot = nc.dram_tensor("wot", [128, D], BF16, kind="ExternalInput")
    cos2 = nc.dram_tensor("cos2", [128, 2, S], BF16, kind="ExternalInput")
    sinp2 = nc.dram_tensor("sinp2", [128, 2, S], BF16, kind="ExternalInput")
    pswap = nc.dram_tensor("pswap", [128, 128], BF16, kind="ExternalInput")
    masksq = nc.dram_tensor("masksq", [128, 512], BF16, kind="ExternalInput")
    y = nc.dram_tensor("y", [BS, D], BF16, kind="ExternalOutput")

    with tile.TileContext(nc) as tc:
        with tc.tile_pool(name="const", bufs=1) as constp, \
             tc.tile_pool(name="xt", bufs=4) as xtp, \
             tc.tile_pool(name="qk", bufs=2) as qkp, \
             tc.tile_pool(name="vsb", bufs=2) as vsp, \
             tc.tile_pool(name="u", bufs=4) as up, \
             tc.tile_pool(name="ptile", bufs=34) as pp, \
             tc.tile_pool(name="otp", bufs=2) as otp, \
             tc.tile_pool(name="rc", bufs=4) as rcp, \
             tc.tile_pool(name="yout", bufs=7) as yop, \
             tc.tile_pool(name="psum", bufs=1, space="PSUM") as psp:

            # ---- constant tiles (DMAs emitted in the prologue below) ----
            wq_sb = constp.tile([128, 8, 128], BF16)
            wk_sb = constp.tile([128, 8, 128], BF16)
            sinp_a = constp.tile([128, 2, 512], BF16)
            cos_a = constp.tile([128, 2, 512], BF16)
            sinp_b = constp.tile([128, 2, S - 512], BF16)
            cos_b = constp.tile([128, 2, S - 512], BF16)
            wv_sb = constp.tile([128, 8, 128], BF16)
            psw_sb = constp.tile([128, 128], BF16)
            msq_sb = constp.tile([128, 512], BF16)
            wot_sb = constp.tile([128, D], BF16)
            warm = constp.tile([128, 2], F32)

            # ---- per-batch state (bufs=2 pools ring across batches) ----
            state = {}

            def xt_load(b, c, eng=None):
                tb0 = (b % B) * S + 512 * c
                xt = xtp.tile([128, 8, 512], BF16, tag="xt", name=f"xt{b}_{c}")
                (eng or nc.sync).dma_start(out=xt, in_=xbt[:, :, tb0:tb0 + 512])
                state[("xt", b, c)] = xt

            def proj_qk(b, c):
                """PE: 16 qk mms. DVE: u/cc rope muls (emitted here so they
                drain the proj-tag psum ring early)."""
                if ("qtr", b) not in state:
                    state[("qtr", b)] = qkp.tile([128, S], BF16, tag="qtr", name=f"qtr{b}")
                    state[("ktr", b)] = qkp.tile([128, S], BF16, tag="ktr", name=f"ktr{b}")
                    v = vsp.tile([128, 16, 192], BF16, tag="v", name=f"v{b}")
                    nc.gpsimd.memset(v[:, :, 64:128], 1.0)
                    state[("v", b)] = v
                t0 = 512 * c
                xt = state[("xt", b, c)]
                qk_ps = []
                for a, w_sb in ((0, wq_sb), (1, wk_sb)):
                    ps = psp.tile([128, 512], F32, tag="proj", bufs=2, name=f"qk{b}_{c}_{a}")
                    for it in range(8):
                        nc.tensor.matmul(ps, w_sb[:, it, :], xt[:, it, :],
                                         start=(it == 0), stop=(it == 7),
                                         _label=f"proj{'QK'[a]} b{b} c{c} it{it}")
                    qk_ps.append(ps)
                u_sb = up.tile([128, 2, 512], BF16, tag="u")
                cc_sb = up.tile([128, 2, 512], BF16, tag="cc")
                sinp_t = sinp_a if c == 0 else sinp_b[:, :, t0 - 512:t0]
                cos_t = cos_a if c == 0 else cos_b[:, :, t0 - 512:t0]
                for a in range(2):
                    nc.vector.tensor_mul(u_sb[:, a, :], qk_ps[a], sinp_t[:, a, :])
                    nc.vector.tensor_mul(cc_sb[:, a, :], qk_ps[a], cos_t[:, a, :])
                state[("ucc", b, c)] = (u_sb, cc_sb)

            def proj_vr(b, c):
                """PE: 32 v mms + 2 swap mms. DVE: rope adds. ACT: v copy."""
                v_sb = state[("v", b)]
                qk_dst = (state[("qtr", b)], state[("ktr", b)])
                t0 = 512 * c
                xt = state.pop(("xt", b, c))
                u_sb, cc_sb = state.pop(("ucc", b, c))
                vt_ps = psp.tile([128, 512], F32, tag="proj", bufs=2)
                for tt in range(4):
                    for it in range(8):
                        nc.tensor.matmul(vt_ps[:, 128 * tt:128 * tt + 128],
                                         xt[:, it, 128 * tt:128 * tt + 128],
                                         wv_sb[:, it, :],
                                         start=(it == 0), stop=(it == 7),
                                         _label=f"projV b{b} c{c} t{tt} it{it}")
                for a in range(2):
                    if a == 0:
                        sw_ps = psp.tile([128, 512], F32, tag="pv", bufs=1, name=f"sw{b}_{c}_{a}")
                    else:
                        sw_ps = psp.tile([128, 512], F32, tag="proj", bufs=2, name=f"sw{b}_{c}_{a}")
                    nc.tensor.matmul(sw_ps, psw_sb, u_sb[:, a, :],
                                     start=True, stop=True, _label=f"swap b{b} c{c} a{a}")
                    nc.vector.tensor_add(qk_dst[a][:, t0:t0 + 512], sw_ps, cc_sb[:, a, :])
                # v: [tok%128, tt, hd] -> v_sb ktiles [VA(64) | ones | VB(64)]
                vv = vt_ps.rearrange("p (t c) -> p t c", t=4)
                nc.vector.tensor_copy(v_sb[:, 4 * c:4 * c + 4, 0:64], vv[:, :, 0:64])
                nc.vector.tensor_copy(v_sb[:, 4 * c:4 * c + 4, 128:192], vv[:, :, 64:128])

            def proj_chunk(b, c):
                proj_qk(b, c)
                proj_vr(b, c)

            def scores_block(b, qc):
                """PE: scores mms (trimmed on diagonal). ACT: exps. DVE: masks."""
                qtr, ktr = state[("qtr", b)], state[("ktr", b)]
                q0 = 512 * qc
                for kp in range(2 * (qc + 1)):
                    diag = kp >= 2 * qc
                    for hh in range(2):
                        h0 = 64 * hh
                        p_t = pp.tile([128, 1024], BF16, tag="p", name=f"p{b}_{qc}_{kp}_{hh}")
                        st = psp.tile([128, 1024], F32, tag="st", bufs=2, name=f"st{b}_{qc}_{kp}_{hh}")
                        for j in range(2):
                            ki = 2 * kp + j
                            d = ki - 4 * qc
                            trim = 128 * d if diag else 0
                            nc.tensor.matmul(
                                st[:, 512 * j + trim:512 * j + 512],
                                ktr[h0:h0 + 64, 128 * ki:128 * ki + 128],
                                qtr[h0:h0 + 64, q0 + trim:q0 + 512],
                                start=True, stop=True,
                                _label=f"score b{b} q{qc} kp{kp} h{hh} j{j}")
                            if diag:
                                nc.scalar.activation(
                                    p_t[:, 512 * j + trim:512 * j + 512],
                                    st[:, 512 * j + trim:512 * j + 512],
                                    AF.Exp, scale=0.125)
                                nc.vector.tensor_mul(
                                    p_t[:, 512 * j + trim:512 * j + 512],
                                    p_t[:, 512 * j + trim:512 * j + 512],
                                    msq_sb[:, 0:512 - trim])
                        if not diag:
                            nc.scalar.activation(p_t, st, AF.Exp, scale=0.125)
                        state[("p", b, qc, kp, hh)] = p_t

            def pv_block(b, qc):
                """PE: pv mms. DVE: 2 recips + 2 muls -> ot."""
                if ("ot", b) not in state:
                    state[("ot", b)] = otp.tile([128, S], BF16, tag="ot", name=f"ot{b}")
                ot = state[("ot", b)]
                v_sb = state[("v", b)]
                q0 = 512 * qc
                nk = 4 * qc + 4
                pv = psp.tile([128, 1024], F32, tag="pv", bufs=1, name=f"pv{b}_{qc}")
                for hh in range(2):
                    col0 = 0 if hh == 0 else 64
                    for ki in range(nk):
                        d = ki - 4 * qc
                        trim = 128 * d if d >= 0 else 0
                        kp, j = divmod(ki, 2)
                        p_t = state[("p", b, qc, kp, hh)]
                        nc.tensor.matmul(
                            pv[:, 512 * hh + trim:512 * hh + 512],
                            v_sb[:, ki, col0:col0 + 128],
                            p_t[:, 512 * j + trim:512 * j + 512],
                            start=(ki == 0), stop=(ki == nk - 1),
                            _label=f"pv b{b} q{qc} h{hh} ki{ki}")
                for kp in range(2 * (qc + 1)):
                    for hh in range(2):
                        state.pop(("p", b, qc, kp, hh))
                # denominators sit replicated in the ones-rows:
                #   hh=0: rows 64:128 ; hh=1: rows 0:64
                s2 = rcp.tile([128, 512], F32, tag="s2")
                nc.vector.tensor_copy(s2[0:64, :], pv[64:128, 0:512])
                nc.vector.tensor_copy(s2[64:128, :], pv[0:64, 512:1024])
                r2 = rcp.tile([128, 512], F32, tag="r2")
                nc.vector.reciprocal(r2, s2)
                nc.vector.tensor_mul(ot[0:64, q0:q0 + 512], pv[0:64, 0:512], r2[0:64, :])
                nc.vector.tensor_mul(ot[64:128, q0:q0 + 512], pv[64:128, 512:1024], r2[64:128, :])

            def oproj(b, tts, alt=False):
                """PE: 2 mms per t-tile into one wide tile; alternating
                ACT/DVE wide copies. DMA y."""
                ot = state[("ot", b)]
                tb0 = (b % B) * S
                for tt in tts:
                    yo = yop.tile([128, 1024], BF16, tag="yo")
                    if alt and tt % 2 == 1:
                        op_ps = psp.tile([128, 1024], F32, tag="pv", bufs=1, name=f"op{b}_{tt}")
                    else:
                        op_ps = psp.tile([128, 1024], F32, tag="st", bufs=2, name=f"op{b}_{tt}")
                    for oc in range(2):
                        nc.tensor.matmul(op_ps[:, 512 * oc:512 * oc + 512],
                                         ot[:, 128 * tt:128 * tt + 128],
                                         wot_sb[:, 512 * oc:512 * oc + 512],
                                         start=True, stop=True,
                                         _label=f"oproj b{b} t{tt} o{oc}")
                    if tt % 2 == 0:
                        nc.scalar.activation(yo, op_ps, AF.Copy)
                    else:
                        nc.vector.tensor_copy(yo, op_ps)
                    if tt % 2 == 0:
                        nc.gpsimd.dma_start(out=y[tb0 + 128 * tt:tb0 + 128 * tt + 128, :], in_=yo)
                    else:
                        nc.sync.dma_start(out=y[tb0 + 128 * tt:tb0 + 128 * tt + 128, :], in_=yo)

            def release(b):
                state.pop(("qtr", b))
                state.pop(("ktr", b))
                state.pop(("v", b))
                state.pop(("ot", b))

            # ---- pipelined emission ----
            # steady state per batch n (prev = n-1):
            #  [S2 V1] [P0n S3 V2] [V3 P1n] [O P2n] [P3n S0n] [S1n V0n]
            # prologue: first x chunk + weights win the DMA pipe in
            # need-order (all on the ACT queue so FIFO order is exact),
            # remaining constants trail on gpsimd/SP queues.
            xt_load(0, 0, eng=nc.scalar)
            xt_load(0, 1, eng=nc.scalar)
            xt_load(0, 2, eng=nc.sync)
            xt_load(0, 3, eng=nc.sync)
            nc.scalar.dma_start(out=wq_sb, in_=wqt[:, :, :])
            nc.scalar.dma_start(out=wk_sb, in_=wkt[:, :, :])
            nc.scalar.dma_start(out=wv_sb, in_=wvt[:, :, :])
            nc.scalar.dma_start(out=sinp_a, in_=sinp2[:, :, 0:512])
            nc.scalar.dma_start(out=cos_a, in_=cos2[:, :, 0:512])
            nc.gpsimd.dma_start(out=psw_sb, in_=pswap[:, :])
            nc.gpsimd.dma_start(out=msq_sb, in_=masksq[:, :])
            nc.sync.dma_start(out=sinp_b, in_=sinp2[:, :, 512:S])
            nc.sync.dma_start(out=cos_b, in_=cos2[:, :, 512:S])
            nc.gpsimd.dma_start(out=wot_sb, in_=wot[:, :])
            nc.scalar.activation(warm, psw_sb[:, 0:2], AF.Exp)

            # merged pipeline: per batch-cycle, interleave attn(b) blocks
            # with proj(b+1) sections and oproj(b-1) pairs so every engine
            # sees a mixed diet continuously.
            proj_chunk(0, 0)
            proj_chunk(0, 1)
            scores_block(0, 0)
            proj_chunk(0, 2)
            scores_block(0, 1)
            pv_block(0, 0)
            proj_chunk(0, 3)
            scores_block(0, 2)
            pv_block(0, 1)

            def osec(b, ts, alt=False):
                if b is not None and b >= 0:
                    oproj(b, ts, alt=alt)

            # steady cycles: cycle b finishes attn(b), runs proj(b+1),
            # starts attn(b+1) through qc2/V1, and drains oproj(b-1)/oproj(b).
            for b in range(nb):
                n = b + 1 if b + 1 < nb else None
                prv = b - 1 if b > 0 else None
                if n is not None:
                    xt_load(n, 0)
                    xt_load(n, 1)
                    scores_block(b, 3)
                    pv_block(b, 2)
                    proj_qk(n, 0)
                    osec(prv, range(12, 16))
                    if prv is not None:
                        release(prv)
                    proj_vr(n, 0)
                    pv_block(b, 3)
                    xt_load(n, 2)
                    proj_qk(n, 1)
                    osec(b, range(0, 2))
                    proj_vr(n, 1)
                    scores_block(n, 0)
                    xt_load(n, 3)
                    proj_qk(n, 2)
                    osec(b, range(2, 6))
                    proj_vr(n, 2)
                    scores_block(n, 1)
                    pv_block(n, 0)
                    proj_qk(n, 3)
                    osec(b, range(6, 10))
                    proj_vr(n, 3)
                    scores_block(n, 2)
                    pv_block(n, 1)
                    osec(b, range(10, 12))
                else:
                    # last batch: spread its own o_proj through its attn
                    scores_block(b, 3)
                    osec(prv, range(12, 14))
                    osec(b, range(0, 2))
                    osec(prv, range(14, 16))
                    if prv is not None:
                        release(prv)
                    pv_block(b, 2)
                    osec(b, range(2, 5))
                    osec(b, range(5, 8))
                    pv_block(b, 3)
                    osec(b, range(8, 12))
                    oproj(b, range(12, 16))
                    release(b)

    nc.compile()
    return nc


_NC_CACHE = {}


def _get_nc(nb=B):
    if nb not in _NC_CACHE:
        _NC_CACHE[nb] = build_nc(nb)
    return _NC_CACHE[nb]


def _host_prep(x, Wq, Wk, Wv, Wo):
    x2 = np.ascontiguousarray(x.reshape(BS, D)).astype(BFNP)
    xbt = np.ascontiguousarray(x2.reshape(BS, 8, 128).transpose(2, 1, 0))

    half = 32
    inv_freq = 1.0 / (ROPE_THETA ** (np.arange(half, dtype=np.float64) / half))
    freqs = np.arange(S, dtype=np.float64)[:, None] * inv_freq[None, :]
    c_ = np.cos(freqs).astype(np.float32).T      # [32, S]
    s_ = np.sin(freqs).astype(np.float32).T
    cos1 = np.tile(c_, (4, 1))                        # [128, S]
    sins1 = np.vstack([-s_, s_, -s_, s_])             # [128, S]

    perm = np.zeros(128, dtype=np.int64)
    partner = np.zeros(128, dtype=np.int64)
    for hh in range(2):
        for j in range(64):
            perm[64 * hh + j] = 64 * hh + (2 * j if j < 32 else 2 * (j - 32) + 1)
            partner[64 * hh + j] = 64 * hh + (j + 32) % 64
    pswap = np.zeros((128, 128), dtype=np.float32)
    pswap[partner, np.arange(128)] = 1.0

    sinp1 = sins1[partner]                            # u = ps * sinp trick
    cos2 = np.ascontiguousarray(
        np.broadcast_to(cos1[:, None, :], (128, 2, S))).astype(BFNP)
    sinp2 = np.ascontiguousarray(
        np.broadcast_to(sinp1[:, None, :], (128, 2, S))).astype(BFNP)

    # maskw[p, j] = 1 if j >= p else 0, width 512 (cols >=128 all ones);
    # sliced to the exact exp'd range of each diagonal tile
    jj = np.arange(512)
    masksq = (jj[None, :] >= np.arange(128)[:, None]).astype(np.float32).astype(BFNP)

    in_maps = []
    for c in range(NCORES):
        sl = slice(128 * c, 128 * c + 128)
        in_maps.append({
            "xbt": xbt,
            "wqt": np.ascontiguousarray(
                Wq[sl][perm].T.reshape(8, 128, 128).transpose(1, 0, 2)).astype(BFNP),
            "wkt": np.ascontiguousarray(
                Wk[sl][perm].T.reshape(8, 128, 128).transpose(1, 0, 2)).astype(BFNP),
            "wvt": np.ascontiguousarray(
                Wv[sl].T.reshape(8, 128, 128).transpose(1, 0, 2)).astype(BFNP),
            "wot": np.ascontiguousarray(Wo[:, sl].T).astype(BFNP),
            "cos2": cos2,
            "sinp2": sinp2,
            "pswap": pswap.astype(BFNP),
            "masksq": masksq,
        })
    return in_maps


def kernel(x, Wq, Wk, Wv, Wo):
    global LAST_RESULTS
    x = np.asarray(x, dtype=np.float32)
    Wq = np.asarray(Wq, dtype=np.float32)
    Wk = np.asarray(Wk, dtype=np.float32)
    Wv = np.asarray(Wv, dtype=np.float32)
    Wo = np.asarray(Wo, dtype=np.float32)

    nc = _get_nc(B)
    in_maps = _host_prep(x, Wq, Wk, Wv, Wo)
    res = run_bass_kernel_spmd(nc, in_maps, core_ids=list(range(NCORES)),
                               trace=TRACE)
    LAST_RESULTS = res
    out = np.zeros((BS, D), dtype=np.float32)
    for c in range(NCORES):
        out += np.asarray(res.results[c]["y"]).astype(np.float32)
    return out.reshape(B, S, D)



# revision 9
# speedup vs baseline: 1.0703x; 1.0703x over previous
"""Causal MHA with RoPE on 8 Trainium2 NeuronCores.

Sharding: tensor-parallel over heads. Core c owns heads {2c, 2c+1} (a 128-wide
slice of the model dim). Each core computes Q/K/V projections for its heads,
full causal attention, and a partial o_proj; the host sums the 8 partial
outputs (the "all-reduce").

v3 (on top of the pipelined v2):
  - Q/K/V projections run in fp8e4 DoubleRow mode (4x PE throughput per
    column): host ships x and W as hi+lo fp8 pairs at a common power-of-2
    scale (sx*sw = 2^13); the 3-term product (xh*Wh + xl*Wh + xh*Wl) restores
    ~bf16 accuracy. The 2^13 descale is folded into the host RoPE tables for
    q/k and into the ones-rows value (denominator trick) for v.
  - softmax reciprocals read the PSUM ones-rows directly with
    partition-offset APs (no staging copies).
  - o_proj PSUM->SBUF copies run on the idle Pool engine; y DMAs issue from
    the SP queue.
  - prologue DMAs ordered so the first projection matmul starts ~3us in.

Device layouts (per core):
  x.T   [128 i-part, 8 i-tile, t]  fp8 hi+lo
  qk_sb [128 hd, 2(q/k), t] bf16;  hd = [head A (ev 0:32, od 32:64), head B]
  scores S.T [k, q] per 128-key tile; P = exp(S.T/8) bf16 in SBUF
  PV: v_sb k-tiles [VA(64) | SCALE-rows(64) | VB(64)]; scale-rows give denoms
  o_proj: ot [128 hd, t] bf16 (stationary) x Wo.T [128 hd, 1024] -> y bf16
"""
import sys
sys.path.insert(0, '/opt/trn_rl_repo')

import numpy as np
import ml_dtypes

import concourse.bass as bass
from concourse import bacc
import concourse.mybir as mybir
import concourse.tile as tile
from concourse.bass_utils import run_bass_kernel_spmd

BFNP = ml_dtypes.bfloat16
F8NP = ml_dtypes.float8_e4m3
F32 = mybir.dt.float32
BF16 = mybir.dt.bfloat16
FP8 = mybir.dt.float8e4
DR = mybir.MatmulPerfMode.DoubleRow
AF = mybir.ActivationFunctionType

B, S, D = 4, 2048, 1024
NCORES = 8
BS = B * S
ROPE_THETA = 10000.0

SX = 16.0          # x fp8 scale
SW = 512.0         # W fp8 scale
SCALE = SX * SW    # folded out via rope tables (q,k) and ones-rows (v)

TRACE = False
LAST_RESULTS = None
PE_LABELS = []
PE_LABEL_BY_NAME = {}


def build_nc(nb=B):
    global PE_LABELS
    PE_LABELS = []
    nc = bacc.Bacc()

    _mm = nc.tensor.matmul
    def _mm_tagged(*a, _label="?", **k):
        PE_LABELS.append(_label)
        r = _mm(*a, **k)
        PE_LABEL_BY_NAME[r.ins.name] = _label
        return r
    nc.tensor.matmul = _mm_tagged
    xbth = nc.dram_tensor("xbth", [128, 8, BS], FP8, kind="ExternalInput")
    xbtl = nc.dram_tensor("xbtl", [128, 8, BS], FP8, kind="ExternalInput")
    wqth = nc.dram_tensor("wqth", [128, 8, 128], FP8, kind="ExternalInput")
    wqtl = nc.dram_tensor("wqtl", [128, 8, 128], FP8, kind="ExternalInput")
    wkth = nc.dram_tensor("wkth", [128, 8, 128], FP8, kind="ExternalInput")
    wktl = nc.dram_tensor("wktl", [128, 8, 128], FP8, kind="ExternalInput")
    wvth = nc.dram_tensor("wvth", [128, 8, 128], FP8, kind="ExternalInput")
    wvtl = nc.dram_tensor("wvtl", [128, 8, 128], FP8, kind="ExternalInput")
    wot = nc.dram_tensor("wot", [128, D], BF16, kind="ExternalInput")
    cos2 = nc.dram_tensor("cos2", [128, 2, S], BF16, kind="ExternalInput")
    sinp2 = nc.dram_tensor("sinp2", [128, 2, S], BF16, kind="ExternalInput")
    pswap = nc.dram_tensor("pswap", [128, 128], BF16, kind="ExternalInput")
    masksq = nc.dram_tensor("masksq", [128, 512], BF16, kind="ExternalInput")
    y = nc.dram_tensor("y", [BS, D], BF16, kind="ExternalOutput")

    with tile.TileContext(nc) as tc:
        with tc.tile_pool(name="const", bufs=1) as constp, \
             tc.tile_pool(name="xt", bufs=4) as xtp, \
             tc.tile_pool(name="qk", bufs=2) as qkp, \
             tc.tile_pool(name="vsb", bufs=2) as vsp, \
             tc.tile_pool(name="u", bufs=4) as up, \
             tc.tile_pool(name="ptile", bufs=34) as pp, \
             tc.tile_pool(name="otp", bufs=2) as otp, \
             tc.tile_pool(name="rc", bufs=4) as rcp, \
             tc.tile_pool(name="yout", bufs=7) as yop, \
             tc.tile_pool(name="psum", bufs=1, space="PSUM") as psp:

            # ---- constant tiles (DMAs emitted in the prologue below) ----
            wq_h = constp.tile([128, 8, 128], FP8)
            wq_l = constp.tile([128, 8, 128], FP8)
            wk_h = constp.tile([128, 8, 128], FP8)
            wk_l = constp.tile([128, 8, 128], FP8)
            wv_h = constp.tile([128, 8, 128], FP8)
            wv_l = constp.tile([128, 8, 128], FP8)
            sinp_a = constp.tile([128, 2, 512], BF16)
            cos_a = constp.tile([128, 2, 512], BF16)
            sinp_b = constp.tile([128, 2, S - 512], BF16)
            cos_b = constp.tile([128, 2, S - 512], BF16)
            psw_sb = constp.tile([128, 128], BF16)
            msq_sb = constp.tile([128, 512], BF16)
            wot_sb = constp.tile([128, D], BF16)
            warm = constp.tile([128, 2], F32)

            # ---- per-batch state (bufs=2 pools ring across batches) ----
            state = {}

            def xt_load(b, c, eng=None, enl=None):
                tb0 = (b % B) * S + 512 * c
                xth = xtp.tile([128, 8, 512], FP8, tag="xth", name=f"xth{b}_{c}")
                xtl = xtp.tile([128, 8, 512], FP8, tag="xtl", name=f"xtl{b}_{c}")
                (eng or nc.sync).dma_start(out=xth, in_=xbth[:, :, tb0:tb0 + 512])
                (enl or eng or nc.sync).dma_start(out=xtl, in_=xbtl[:, :, tb0:tb0 + 512])
                state[("xt", b, c)] = (xth, xtl)

            def proj_qk(b, c):
                """PE: 24 DoubleRow qk mms. DVE: u/cc rope muls (emitted here
                so they drain the proj-tag psum ring early)."""
                if ("qtr", b) not in state:
                    state[("qtr", b)] = qkp.tile([128, S], BF16, tag="qtr", name=f"qtr{b}")
                    state[("ktr", b)] = qkp.tile([128, S], BF16, tag="ktr", name=f"ktr{b}")
                    v = vsp.tile([128, 16, 192], BF16, tag="v", name=f"v{b}")
                    nc.gpsimd.memset(v[:, :, 64:128], SCALE)
                    state[("v", b)] = v
                t0 = 512 * c
                xth, xtl = state[("xt", b, c)]
                qk_ps = []
                for a, w_h, w_l in ((0, wq_h, wq_l), (1, wk_h, wk_l)):
                    ps = psp.tile([128, 512], F32, tag="proj", bufs=2, name=f"qk{b}_{c}_{a}")
                    nmm = 0
                    for xs, ws in ((xth, w_h), (xtl, w_h), (xth, w_l)):
                        for m in range(4):
                            nc.tensor.matmul(ps, ws[:, 2 * m:2 * m + 2, :],
                                             xs[:, 2 * m:2 * m + 2, :],
                                             start=(nmm == 0), stop=(nmm == 11),
                                             perf_mode=DR,
                                             _label=f"proj{'QK'[a]} b{b} c{c} m{nmm}")
                            nmm += 1
                    qk_ps.append(ps)
                u_sb = up.tile([128, 2, 512], BF16, tag="u")
                cc_sb = up.tile([128, 2, 512], BF16, tag="cc")
                sinp_t = sinp_a if c == 0 else sinp_b[:, :, t0 - 512:t0]
                cos_t = cos_a if c == 0 else cos_b[:, :, t0 - 512:t0]
                for a in range(2):
                    nc.vector.tensor_mul(u_sb[:, a, :], qk_ps[a], sinp_t[:, a, :])
                    nc.vector.tensor_mul(cc_sb[:, a, :], qk_ps[a], cos_t[:, a, :])
                state[("ucc", b, c)] = (u_sb, cc_sb)

            def proj_vr(b, c):
                """PE: 48 DoubleRow v mms + 2 swap mms. DVE: rope adds +
                v copies."""
                v_sb = state[("v", b)]
                qk_dst = (state[("qtr", b)], state[("ktr", b)])
                t0 = 512 * c
                xth, xtl = state.pop(("xt", b, c))
                u_sb, cc_sb = state.pop(("ucc", b, c))
                vt_ps = psp.tile([128, 512], F32, tag="proj", bufs=2)
                for tt in range(4):
                    nmm = 0
                    for xs, ws in ((xth, wv_h), (xtl, wv_h), (xth, wv_l)):
                        for m in range(4):
                            nc.tensor.matmul(vt_ps[:, 128 * tt:128 * tt + 128],
                                             xs[:, 2 * m:2 * m + 2, 128 * tt:128 * tt + 128],
                                             ws[:, 2 * m:2 * m + 2, :],
                                             start=(nmm == 0), stop=(nmm == 11),
                                             perf_mode=DR,
                                             _label=f"projV b{b} c{c} t{tt} m{nmm}")
                            nmm += 1
                for a in range(2):
                    if a == 0:
                        sw_ps = psp.tile([128, 512], F32, tag="pv", bufs=1, name=f"sw{b}_{c}_{a}")
                    else:
                        sw_ps = psp.tile([128, 512], F32, tag="proj", bufs=2, name=f"sw{b}_{c}_{a}")
                    nc.tensor.matmul(sw_ps, psw_sb, u_sb[:, a, :],
                                     start=True, stop=True, _label=f"swap b{b} c{c} a{a}")
                    nc.vector.tensor_add(qk_dst[a][:, t0:t0 + 512], sw_ps, cc_sb[:, a, :])
                # v: [tok%128, tt, hd] -> v_sb ktiles [VA(64) | SCALE | VB(64)]
                vv = vt_ps.rearrange("p (t c) -> p t c", t=4)
                nc.vector.tensor_copy(v_sb[:, 4 * c:4 * c + 4, 0:64], vv[:, :, 0:64])
                nc.vector.tensor_copy(v_sb[:, 4 * c:4 * c + 4, 128:192], vv[:, :, 64:128])

            def proj_chunk(b, c):
                proj_qk(b, c)
                proj_vr(b, c)

            def scores_block(b, qc):
                """PE: scores mms (trimmed on diagonal). ACT: exps. DVE: masks."""
                qtr, ktr = state[("qtr", b)], state[("ktr", b)]
                q0 = 512 * qc
                for kp in range(2 * (qc + 1)):
                    diag = kp >= 2 * qc
                    for hh in range(2):
                        h0 = 64 * hh
                        p_t = pp.tile([128, 1024], BF16, tag="p", name=f"p{b}_{qc}_{kp}_{hh}")
                        st = psp.tile([128, 1024], F32, tag="st", bufs=2, name=f"st{b}_{qc}_{kp}_{hh}")
                        for j in range(2):
                            ki = 2 * kp + j
                            d = ki - 4 * qc
                            trim = 128 * d if diag else 0
                            nc.tensor.matmul(
                                st[:, 512 * j + trim:512 * j + 512],
                                ktr[h0:h0 + 64, 128 * ki:128 * ki + 128],
                                qtr[h0:h0 + 64, q0 + trim:q0 + 512],
                                start=True, stop=True,
                                _label=f"score b{b} q{qc} kp{kp} h{hh} j{j}")
                            if diag:
                                nc.scalar.activation(
                                    p_t[:, 512 * j + trim:512 * j + 512],
                                    st[:, 512 * j + trim:512 * j + 512],
                                    AF.Exp, scale=0.125)
                                # mask only bites in the first 128 cols
                                # (jcol >= p is trivially true beyond)
                                nc.gpsimd.tensor_mul(
                                    p_t[:, 512 * j + trim:512 * j + trim + 128],
                                    p_t[:, 512 * j + trim:512 * j + trim + 128],
                                    msq_sb[:, 0:128])
                        if not diag:
                            nc.scalar.activation(p_t, st, AF.Exp, scale=0.125)
                        state[("p", b, qc, kp, hh)] = p_t

            def pv_block(b, qc):
                """PE: pv mms. DVE: 2 recips + 2 muls -> ot."""
                if ("ot", b) not in state:
                    state[("ot", b)] = otp.tile([128, S], BF16, tag="ot", name=f"ot{b}")
                ot = state[("ot", b)]
                v_sb = state[("v", b)]
                q0 = 512 * qc
                nk = 4 * qc + 4
                pv = psp.tile([128, 1024], F32, tag="pv", bufs=1, name=f"pv{b}_{qc}")
                for hh in range(2):
                    col0 = 0 if hh == 0 else 64
                    for ki in range(nk):
                        d = ki - 4 * qc
                        trim = 128 * d if d >= 0 else 0
                        kp, j = divmod(ki, 2)
                        p_t = state[("p", b, qc, kp, hh)]
                        nc.tensor.matmul(
                            pv[:, 512 * hh + trim:512 * hh + 512],
                            v_sb[:, ki, col0:col0 + 128],
                            p_t[:, 512 * j + trim:512 * j + 512],
                            start=(ki == 0), stop=(ki == nk - 1),
                            _label=f"pv b{b} q{qc} h{hh} ki{ki}")
                for kp in range(2 * (qc + 1)):
                    for hh in range(2):
                        state.pop(("p", b, qc, kp, hh))
                # denominators sit replicated in the ones-rows:
                #   hh=0: rows 64:128 ; hh=1: rows 0:64
                r2 = rcp.tile([128, 512], F32, tag="r2")
                nc.vector.reciprocal(r2[0:64, :], pv[64:128, 0:512])
                nc.vector.reciprocal(r2[64:128, :], pv[0:64, 512:1024])
                nc.vector.tensor_mul(ot[0:64, q0:q0 + 512], pv[0:64, 0:512], r2[0:64, :])
                nc.vector.tensor_mul(ot[64:128, q0:q0 + 512], pv[64:128, 512:1024], r2[64:128, :])

            def oproj(b, tts, alt=False):
                """PE: 2 mms per t-tile into one wide tile; mostly-DVE wide
                copies (ACT every 4th). DMA y on SP queue."""
                ot = state[("ot", b)]
                tb0 = (b % B) * S
                for tt in tts:
                    yo = yop.tile([128, 1024], BF16, tag="yo")
                    if alt and tt % 2 == 1:
                        op_ps = psp.tile([128, 1024], F32, tag="pv", bufs=1, name=f"op{b}_{tt}")
                    else:
                        op_ps = psp.tile([128, 1024], F32, tag="st", bufs=2, name=f"op{b}_{tt}")
                    for oc in range(2):
                        nc.tensor.matmul(op_ps[:, 512 * oc:512 * oc + 512],
                                         ot[:, 128 * tt:128 * tt + 128],
                                         wot_sb[:, 512 * oc:512 * oc + 512],
                                         start=True, stop=True,
                                         _label=f"oproj b{b} t{tt} o{oc}")
                    if tt % 4 == 0:
                        nc.scalar.activation(yo, op_ps, AF.Copy)
                    else:
                        nc.vector.tensor_copy(yo, op_ps)
                    nc.sync.dma_start(out=y[tb0 + 128 * tt:tb0 + 128 * tt + 128, :], in_=yo)

            def release(b):
                state.pop(("qtr", b))
                state.pop(("ktr", b))
                state.pop(("v", b))
                state.pop(("ot", b))

            # ---- pipelined emission ----
            # steady state per batch n (prev = n-1):
            #  [S2 V1] [P0n S3 V2] [V3 P1n] [O P2n] [P3n S0n] [S1n V0n]
            # prologue: the first-chunk working set wins the DMA pipe in exact
            # need-order (q pass deps first), remaining constants trail.
            xth0 = xtp.tile([128, 8, 512], FP8, tag="xth", name="xth0_0")
            xtl0 = xtp.tile([128, 8, 512], FP8, tag="xtl", name="xtl0_0")
            nc.scalar.dma_start(out=wq_h, in_=wqth[:, :, :])
            nc.scalar.dma_start(out=wk_h, in_=wkth[:, :, :])
            nc.scalar.dma_start(out=xth0, in_=xbth[:, :, 0:512])
            nc.scalar.dma_start(out=xtl0, in_=xbtl[:, :, 0:512])
            nc.scalar.dma_start(out=wq_l, in_=wqtl[:, :, :])
            nc.scalar.dma_start(out=wk_l, in_=wktl[:, :, :])
            state[("xt", 0, 0)] = (xth0, xtl0)
            nc.gpsimd.dma_start(out=wv_h, in_=wvth[:, :, :])
            nc.gpsimd.dma_start(out=wv_l, in_=wvtl[:, :, :])
            nc.gpsimd.dma_start(out=psw_sb, in_=pswap[:, :])
            nc.gpsimd.dma_start(out=msq_sb, in_=masksq[:, :])
            nc.sync.dma_start(out=sinp_a, in_=sinp2[:, :, 0:512])
            nc.sync.dma_start(out=cos_a, in_=cos2[:, :, 0:512])
            xt_load(0, 1, eng=nc.scalar)
            xt_load(0, 2, eng=nc.sync)
            nc.sync.dma_start(out=sinp_b, in_=sinp2[:, :, 512:S])
            nc.sync.dma_start(out=cos_b, in_=cos2[:, :, 512:S])
            xt_load(0, 3, eng=nc.sync)
            nc.gpsimd.dma_start(out=wot_sb, in_=wot[:, :])
            nc.scalar.activation(warm, psw_sb[:, 0:2], AF.Exp)

            # merged pipeline: per batch-cycle, interleave attn(b) blocks
            # with proj(b+1) sections and oproj(b-1) pairs so every engine
            # sees a mixed diet continuously.
            proj_chunk(0, 0)
            proj_chunk(0, 1)
            scores_block(0, 0)
            proj_chunk(0, 2)
            scores_block(0, 1)
            pv_block(0, 0)
            proj_chunk(0, 3)
            scores_block(0, 2)
            pv_block(0, 1)

            def osec(b, ts, alt=False):
                if b is not None and b >= 0:
                    oproj(b, ts, alt=alt)

            # steady cycles: cycle b finishes attn(b), runs proj(b+1),
            # starts attn(b+1) through qc2/V1, and drains oproj(b-1)/oproj(b).
            for b in range(nb):
                n = b + 1 if b + 1 < nb else None
                prv = b - 1 if b > 0 else None
                if n is not None:
                    xt_load(n, 0)
                    xt_load(n, 1)
                    scores_block(b, 3)
                    pv_block(b, 2)
                    proj_qk(n, 0)
                    osec(prv, range(12, 16))
                    if prv is not None:
                        release(prv)
                    proj_vr(n, 0)
                    pv_block(b, 3)
                    xt_load(n, 2)
                    proj_qk(n, 1)
                    osec(b, range(0, 2))
                    proj_vr(n, 1)
                    scores_block(n, 0)
                    xt_load(n, 3)
                    proj_qk(n, 2)
                    osec(b, range(2, 6))
                    proj_vr(n, 2)
                    scores_block(n, 1)
                    pv_block(n, 0)
                    proj_qk(n, 3)
                    osec(b, range(6, 10))
                    proj_vr(n, 3)
                    scores_block(n, 2)
                    pv_block(n, 1)
                    osec(b, range(10, 12))
                else:
                    # last batch: spread its own o_proj through its attn
                    scores_block(b, 3)
                    osec(prv, range(12, 14))
                    osec(b, range(0, 2))
                    osec(prv, range(14, 16))
                    if prv is not None:
                        release(prv)
                    pv_block(b, 2)
                    osec(b, range(2, 5))
                    osec(b, range(5, 8))
                    pv_block(b, 3)
                    osec(b, range(8, 12))
                    oproj(b, range(12, 16))
                    release(b)

    nc.compile()
    return nc


_NC_CACHE = {}


def _get_nc(nb=B):
    if nb not in _NC_CACHE:
        _NC_CACHE[nb] = build_nc(nb)
    return _NC_CACHE[nb]


def _f8(a):
    return a.astype(F8NP)


def _host_prep(x, Wq, Wk, Wv, Wo):
    x2 = np.ascontiguousarray(x.reshape(BS, D))
    xs = (x2 * SX).astype(np.float32)
    xh = _f8(xs)
    xl = _f8(xs - xh.astype(np.float32))

    def xbt_layout(a):  # [BS, D] fp8 -> [128, 8, BS]
        return np.ascontiguousarray(a.reshape(BS, 8, 128).transpose(2, 1, 0))

    xbth = xbt_layout(xh)
    xbtl = xbt_layout(xl)

    half = 32
    inv_freq = 1.0 / (ROPE_THETA ** (np.arange(half, dtype=np.float64) / half))
    freqs = np.arange(S, dtype=np.float64)[:, None] * inv_freq[None, :]
    c_ = np.cos(freqs).astype(np.float32).T      # [32, S]
    s_ = np.sin(freqs).astype(np.float32).T
    cos1 = np.tile(c_, (4, 1))                        # [128, S]
    sins1 = np.vstack([-s_, s_, -s_, s_])             # [128, S]

    perm = np.zeros(128, dtype=np.int64)
    partner = np.zeros(128, dtype=np.int64)
    for hh in range(2):
        for j in range(64):
            perm[64 * hh + j] = 64 * hh + (2 * j if j < 32 else 2 * (j - 32) + 1)
            partner[64 * hh + j] = 64 * hh + (j + 32) % 64
    pswap = np.zeros((128, 128), dtype=np.float32)
    pswap[partner, np.arange(128)] = 1.0

    sinp1 = sins1[partner] / SCALE                    # u = ps * sinp trick
    cos1 = cos1 / SCALE                               # fold fp8 descale
    cos2 = np.ascontiguousarray(
        np.broadcast_to(cos1[:, None, :], (128, 2, S))).astype(BFNP)
    sinp2 = np.ascontiguousarray(
        np.broadcast_to(sinp1[:, None, :], (128, 2, S))).astype(BFNP)

    # maskw[p, j] = 1 if j >= p else 0, width 512 (cols >=128 all ones);
    # sliced to the exact exp'd range of each diagonal tile
    jj = np.arange(512)
    masksq = (jj[None, :] >= np.arange(128)[:, None]).astype(np.float32).astype(BFNP)

    def w_hilo(Wsl):  # [128 rows, D] (already permuted/sliced) -> hi,lo [128,8,128]
        ws = (Wsl.T * SW).astype(np.float32)          # [D, 128]
        wh = _f8(ws)
        wl = _f8(ws - wh.astype(np.float32))
        def lay(a):
            return np.ascontiguousarray(a.reshape(8, 128, 128).transpose(1, 0, 2))
        return lay(wh), lay(wl)

    in_maps = []
    for c in range(NCORES):
        sl = slice(128 * c, 128 * c + 128)
        wqh, wql = w_hilo(Wq[sl][perm])
        wkh, wkl = w_hilo(Wk[sl][perm])
        wvh, wvl = w_hilo(Wv[sl])
        in_maps.append({
            "xbth": xbth,
            "xbtl": xbtl,
            "wqth": wqh, "wqtl": wql,
            "wkth": wkh, "wktl": wkl,
            "wvth": wvh, "wvtl": wvl,
            "wot": np.ascontiguousarray(Wo[:, sl].T).astype(BFNP),
            "cos2": cos2,
            "sinp2": sinp2,
            "pswap": pswap.astype(BFNP),
            "masksq": masksq,
        })
    return in_maps


def kernel(x, Wq, Wk, Wv, Wo):
    global LAST_RESULTS
    x = np.asarray(x, dtype=np.float32)
    Wq = np.asarray(Wq, dtype=np.float32)
    Wk = np.asarray(Wk, dtype=np.float32)
    Wv = np.asarray(Wv, dtype=np.float32)
    Wo = np.asarray(Wo, dtype=np.float32)

    nc = _get_nc(B)
    in_maps = _host_prep(x, Wq, Wk, Wv, Wo)
    res = run_bass_kernel_spmd(nc, in_maps, core_ids=list(range(NCORES)),
                               trace=TRACE)
    LAST_RESULTS = res
    out = np.zeros((BS, D), dtype=np.float32)
    for c in range(NCORES):
        out += np.asarray(res.results[c]["y"]).astype(np.float32)
    return out.reshape(B, S, D)


# revision 16
# speedup vs baseline: 1.0785x; 1.0077x over previous
"""Causal MHA with RoPE on 8 Trainium2 NeuronCores.

Sharding: tensor-parallel over heads. Core c owns heads {2c, 2c+1} (a 128-wide
slice of the model dim). Each core computes Q/K/V projections for its heads,
full causal attention, and a partial o_proj; the host sums the 8 partial
outputs (the "all-reduce").

v3 (on top of the pipelined v2):
  - Q/K/V projections run in fp8e4 DoubleRow mode (4x PE throughput per
    column): host ships x and W as hi+lo fp8 pairs at a common power-of-2
    scale (sx*sw = 2^13); the 3-term product (xh*Wh + xl*Wh + xh*Wl) restores
    ~bf16 accuracy. The 2^13 descale is folded into the host RoPE tables for
    q/k and into the ones-rows value (denominator trick) for v.
  - softmax reciprocals read the PSUM ones-rows directly with
    partition-offset APs (no staging copies).
  - o_proj PSUM->SBUF copies run on the idle Pool engine; y DMAs issue from
    the SP queue.
  - prologue DMAs ordered so the first projection matmul starts ~3us in.

Device layouts (per core):
  x.T   [128 i-part, 8 i-tile, t]  fp8 hi+lo
  qk_sb [128 hd, 2(q/k), t] bf16;  hd = [head A (ev 0:32, od 32:64), head B]
  scores S.T [k, q] per 128-key tile; P = exp(S.T/8) bf16 in SBUF
  PV: v_sb k-tiles [VA(64) | SCALE-rows(64) | VB(64)]; scale-rows give denoms
  o_proj: ot [128 hd, t] bf16 (stationary) x Wo.T [128 hd, 1024] -> y bf16
"""
import sys
sys.path.insert(0, '/opt/trn_rl_repo')

import numpy as np
import ml_dtypes

import concourse.bass as bass
from concourse import bacc
import concourse.mybir as mybir
import concourse.tile as tile
from concourse.bass_utils import run_bass_kernel_spmd

BFNP = ml_dtypes.bfloat16
F8NP = ml_dtypes.float8_e4m3
F32 = mybir.dt.float32
BF16 = mybir.dt.bfloat16
FP8 = mybir.dt.float8e4
DR = mybir.MatmulPerfMode.DoubleRow
AF = mybir.ActivationFunctionType

B, S, D = 4, 2048, 1024
NCORES = 8
BS = B * S
ROPE_THETA = 10000.0

SX = 16.0          # x fp8 scale
SW = 512.0         # W fp8 scale
SCALE = SX * SW    # folded out via rope tables (q,k) and ones-rows (v)

TRACE = False
LAST_RESULTS = None
PE_LABELS = []
PE_LABEL_BY_NAME = {}


def build_nc(nb=B):
    global PE_LABELS
    PE_LABELS = []
    nc = bacc.Bacc()

    _mm = nc.tensor.matmul
    def _mm_tagged(*a, _label="?", **k):
        PE_LABELS.append(_label)
        r = _mm(*a, **k)
        PE_LABEL_BY_NAME[r.ins.name] = _label
        return r
    nc.tensor.matmul = _mm_tagged
    xbth = nc.dram_tensor("xbth", [128, 8, BS], FP8, kind="ExternalInput")
    xbtl = nc.dram_tensor("xbtl", [128, 8, BS], FP8, kind="ExternalInput")
    wqth = nc.dram_tensor("wqth", [128, 8, 128], FP8, kind="ExternalInput")
    wqtl = nc.dram_tensor("wqtl", [128, 8, 128], FP8, kind="ExternalInput")
    wkth = nc.dram_tensor("wkth", [128, 8, 128], FP8, kind="ExternalInput")
    wktl = nc.dram_tensor("wktl", [128, 8, 128], FP8, kind="ExternalInput")
    wvth = nc.dram_tensor("wvth", [128, 8, 128], FP8, kind="ExternalInput")
    wvtl = nc.dram_tensor("wvtl", [128, 8, 128], FP8, kind="ExternalInput")
    wot = nc.dram_tensor("wot", [128, D], BF16, kind="ExternalInput")
    cos2 = nc.dram_tensor("cos2", [128, 2, S], BF16, kind="ExternalInput")
    sinp2 = nc.dram_tensor("sinp2", [128, 2, S], BF16, kind="ExternalInput")
    pswap = nc.dram_tensor("pswap", [128, 128], BF16, kind="ExternalInput")
    masksq = nc.dram_tensor("masksq", [128, 512], BF16, kind="ExternalInput")
    y = nc.dram_tensor("y", [BS, D], BF16, kind="ExternalOutput")

    with tile.TileContext(nc) as tc:
        with tc.tile_pool(name="const", bufs=1) as constp, \
             tc.tile_pool(name="xt", bufs=4) as xtp, \
             tc.tile_pool(name="qk", bufs=2) as qkp, \
             tc.tile_pool(name="vsb", bufs=2) as vsp, \
             tc.tile_pool(name="u", bufs=4) as up, \
             tc.tile_pool(name="ptile", bufs=34) as pp, \
             tc.tile_pool(name="otp", bufs=2) as otp, \
             tc.tile_pool(name="rc", bufs=4) as rcp, \
             tc.tile_pool(name="yout", bufs=7) as yop, \
             tc.tile_pool(name="psum", bufs=1, space="PSUM") as psp:

            # ---- constant tiles (DMAs emitted in the prologue below) ----
            wq_h = constp.tile([128, 8, 128], FP8)
            wq_l = constp.tile([128, 8, 128], FP8)
            wk_h = constp.tile([128, 8, 128], FP8)
            wk_l = constp.tile([128, 8, 128], FP8)
            wv_h = constp.tile([128, 8, 128], FP8)
            wv_l = constp.tile([128, 8, 128], FP8)
            sinp_a = constp.tile([128, 2, 512], BF16)
            cos_a = constp.tile([128, 2, 512], BF16)
            sinp_b = constp.tile([128, 2, S - 512], BF16)
            cos_b = constp.tile([128, 2, S - 512], BF16)
            psw_sb = constp.tile([128, 128], BF16)
            msq_sb = constp.tile([128, 512], BF16)
            wot_sb = constp.tile([128, D], BF16)
            warm = constp.tile([128, 2], F32)

            # ---- per-batch state (bufs=2 pools ring across batches) ----
            state = {}

            def xt_load(b, c, eng=None, enl=None):
                tb0 = (b % B) * S + 512 * c
                xth = xtp.tile([128, 8, 512], FP8, tag="xth", name=f"xth{b}_{c}")
                xtl = xtp.tile([128, 8, 512], FP8, tag="xtl", name=f"xtl{b}_{c}")
                (eng or nc.sync).dma_start(out=xth, in_=xbth[:, :, tb0:tb0 + 512])
                (enl or eng or nc.sync).dma_start(out=xtl, in_=xbtl[:, :, tb0:tb0 + 512])
                state[("xt", b, c)] = (xth, xtl)

            def proj_qk(b, c):
                """PE: 24 DoubleRow qk mms. DVE: u/cc rope muls (emitted here
                so they drain the proj-tag psum ring early)."""
                if ("qtr", b) not in state:
                    state[("qtr", b)] = qkp.tile([128, S], BF16, tag="qtr", name=f"qtr{b}")
                    state[("ktr", b)] = qkp.tile([128, S], BF16, tag="ktr", name=f"ktr{b}")
                    v = vsp.tile([128, 16, 192], BF16, tag="v", name=f"v{b}")
                    nc.gpsimd.memset(v[:, :, 64:128], SCALE)
                    state[("v", b)] = v
                t0 = 512 * c
                xth, xtl = state[("xt", b, c)]
                qk_ps = []
                for a, w_h, w_l in ((0, wq_h, wq_l), (1, wk_h, wk_l)):
                    ps = psp.tile([128, 512], F32, tag="proj", bufs=2, name=f"qk{b}_{c}_{a}")
                    nmm = 0
                    for xs, ws in ((xth, w_h), (xtl, w_h), (xth, w_l)):
                        for m in range(4):
                            nc.tensor.matmul(ps, ws[:, 2 * m:2 * m + 2, :],
                                             xs[:, 2 * m:2 * m + 2, :],
                                             start=(nmm == 0), stop=(nmm == 11),
                                             perf_mode=DR,
                                             _label=f"proj{'QK'[a]} b{b} c{c} m{nmm}")
                            nmm += 1
                    qk_ps.append(ps)
                u_sb = up.tile([128, 2, 512], BF16, tag="u")
                cc_sb = up.tile([128, 2, 512], BF16, tag="cc")
                sinp_t = sinp_a if c == 0 else sinp_b[:, :, t0 - 512:t0]
                cos_t = cos_a if c == 0 else cos_b[:, :, t0 - 512:t0]
                for a in range(2):
                    nc.vector.tensor_mul(u_sb[:, a, :], qk_ps[a], sinp_t[:, a, :])
                    nc.vector.tensor_mul(cc_sb[:, a, :], qk_ps[a], cos_t[:, a, :])
                state[("ucc", b, c)] = (u_sb, cc_sb)

            def proj_vr(b, c):
                """PE: 48 DoubleRow v mms + 2 swap mms. DVE: rope adds +
                v copies."""
                v_sb = state[("v", b)]
                qk_dst = (state[("qtr", b)], state[("ktr", b)])
                t0 = 512 * c
                xth, xtl = state.pop(("xt", b, c))
                u_sb, cc_sb = state.pop(("ucc", b, c))
                vt_ps = psp.tile([128, 512], F32, tag="proj", bufs=2)
                for tt in range(4):
                    nmm = 0
                    for xs, ws in ((xth, wv_h), (xtl, wv_h), (xth, wv_l)):
                        for m in range(4):
                            nc.tensor.matmul(vt_ps[:, 128 * tt:128 * tt + 128],
                                             xs[:, 2 * m:2 * m + 2, 128 * tt:128 * tt + 128],
                                             ws[:, 2 * m:2 * m + 2, :],
                                             start=(nmm == 0), stop=(nmm == 11),
                                             perf_mode=DR,
                                             _label=f"projV b{b} c{c} t{tt} m{nmm}")
                            nmm += 1
                for a in range(2):
                    if a == 0:
                        sw_ps = psp.tile([128, 512], F32, tag="pv", bufs=1, name=f"sw{b}_{c}_{a}")
                    else:
                        sw_ps = psp.tile([128, 512], F32, tag="proj", bufs=2, name=f"sw{b}_{c}_{a}")
                    nc.tensor.matmul(sw_ps, psw_sb, u_sb[:, a, :],
                                     start=True, stop=True, _label=f"swap b{b} c{c} a{a}")
                    nc.vector.tensor_add(qk_dst[a][:, t0:t0 + 512], sw_ps, cc_sb[:, a, :])
                # v: [tok%128, tt, hd] -> v_sb ktiles [VA(64) | SCALE | VB(64)]
                vv = vt_ps.rearrange("p (t c) -> p t c", t=4)
                nc.vector.tensor_copy(v_sb[:, 4 * c:4 * c + 4, 0:64], vv[:, :, 0:64])
                nc.vector.tensor_copy(v_sb[:, 4 * c:4 * c + 4, 128:192], vv[:, :, 64:128])

            def proj_chunk(b, c):
                proj_qk(b, c)
                proj_vr(b, c)

            def scores_block(b, qc):
                """PE: scores mms (trimmed on diagonal). ACT: exps. DVE: masks."""
                qtr, ktr = state[("qtr", b)], state[("ktr", b)]
                q0 = 512 * qc
                for kp in range(2 * (qc + 1)):
                    diag = kp >= 2 * qc
                    for hh in range(2):
                        h0 = 64 * hh
                        p_t = pp.tile([128, 1024], BF16, tag="p", name=f"p{b}_{qc}_{kp}_{hh}")
                        st = psp.tile([128, 1024], F32, tag="st", bufs=2, name=f"st{b}_{qc}_{kp}_{hh}")
                        for j in range(2):
                            ki = 2 * kp + j
                            d = ki - 4 * qc
                            trim = 128 * d if diag else 0
                            nc.tensor.matmul(
                                st[:, 512 * j + trim:512 * j + 512],
                                ktr[h0:h0 + 64, 128 * ki:128 * ki + 128],
                                qtr[h0:h0 + 64, q0 + trim:q0 + 512],
                                start=True, stop=True,
                                _label=f"score b{b} q{qc} kp{kp} h{hh} j{j}")
                            if diag:
                                nc.scalar.activation(
                                    p_t[:, 512 * j + trim:512 * j + 512],
                                    st[:, 512 * j + trim:512 * j + 512],
                                    AF.Exp, scale=0.125)
                                # mask only bites in the first 128 cols
                                # (jcol >= p is trivially true beyond)
                                nc.gpsimd.tensor_mul(
                                    p_t[:, 512 * j + trim:512 * j + trim + 128],
                                    p_t[:, 512 * j + trim:512 * j + trim + 128],
                                    msq_sb[:, 0:128])
                        if not diag:
                            nc.scalar.activation(p_t, st, AF.Exp, scale=0.125)
                        state[("p", b, qc, kp, hh)] = p_t

            def pv_block(b, qc, cols=(0, 512), keep_p=False, pv=None):
                """PE: pv mms. DVE: 2 recips + 2 muls -> ot.
                cols selects a query sub-range of the 512-wide block (used to
                pipeline the final block against its o_proj)."""
                if ("ot", b) not in state:
                    state[("ot", b)] = otp.tile([128, S], BF16, tag="ot", name=f"ot{b}")
                ot = state[("ot", b)]
                v_sb = state[("v", b)]
                c0, c1 = cols
                q0 = 512 * qc
                nk = 4 * qc + 4
                if pv is None:
                    pv = psp.tile([128, 1024], F32, tag="pv", bufs=1, name=f"pv{b}_{qc}_{c0}")
                for hh in range(2):
                    col0 = 0 if hh == 0 else 64
                    kis = [ki for ki in range(nk)
                           if max(128 * (ki - 4 * qc), c0) < c1]
                    for ii, ki in enumerate(kis):
                        d = ki - 4 * qc
                        trim = max(128 * d if d >= 0 else 0, c0)
                        kp, j = divmod(ki, 2)
                        p_t = state[("p", b, qc, kp, hh)]
                        nc.tensor.matmul(
                            pv[:, 512 * hh + trim:512 * hh + c1],
                            v_sb[:, ki, col0:col0 + 128],
                            p_t[:, 512 * j + trim:512 * j + c1],
                            start=(ii == 0), stop=(ii == len(kis) - 1),
                            _label=f"pv b{b} q{qc} h{hh} ki{ki}")
                if not keep_p:
                    for kp in range(2 * (qc + 1)):
                        for hh in range(2):
                            state.pop(("p", b, qc, kp, hh))
                # denominators sit replicated in the ones-rows:
                #   hh=0: rows 64:128 ; hh=1: rows 0:64
                r2 = rcp.tile([128, 512], F32, tag="r2")
                nc.vector.reciprocal(r2[0:64, c0:c1], pv[64:128, c0:c1])
                nc.vector.reciprocal(r2[64:128, c0:c1], pv[0:64, 512 + c0:512 + c1])
                nc.vector.tensor_mul(ot[0:64, q0 + c0:q0 + c1], pv[0:64, c0:c1],
                                     r2[0:64, c0:c1])
                nc.vector.tensor_mul(ot[64:128, q0 + c0:q0 + c1],
                                     pv[64:128, 512 + c0:512 + c1],
                                     r2[64:128, c0:c1])

            def oproj(b, tts, alt=False, act_copy=None):
                """PE: 2 mms per t-tile into one wide tile; mostly-DVE wide
                copies (ACT every 4th by default). DMA y on SP queue."""
                ot = state[("ot", b)]
                tb0 = (b % B) * S
                for tt in tts:
                    yo = yop.tile([128, 1024], BF16, tag="yo")
                    if alt and tt % 2 == 1:
                        op_ps = psp.tile([128, 1024], F32, tag="pv", bufs=1, name=f"op{b}_{tt}")
                    else:
                        op_ps = psp.tile([128, 1024], F32, tag="st", bufs=2, name=f"op{b}_{tt}")
                    for oc in range(2):
                        nc.tensor.matmul(op_ps[:, 512 * oc:512 * oc + 512],
                                         ot[:, 128 * tt:128 * tt + 128],
                                         wot_sb[:, 512 * oc:512 * oc + 512],
                                         start=True, stop=True,
                                         _label=f"oproj b{b} t{tt} o{oc}")
                    on_act = (tt % 4 == 0) if act_copy is None else act_copy
                    if on_act:
                        nc.scalar.activation(yo, op_ps, AF.Copy)
                    else:
                        nc.vector.tensor_copy(yo, op_ps)
                    nc.sync.dma_start(out=y[tb0 + 128 * tt:tb0 + 128 * tt + 128, :], in_=yo)

            def release(b):
                state.pop(("qtr", b))
                state.pop(("ktr", b))
                state.pop(("v", b))
                state.pop(("ot", b))

            # ---- pipelined emission ----
            # steady state per batch n (prev = n-1):
            #  [S2 V1] [P0n S3 V2] [V3 P1n] [O P2n] [P3n S0n] [S1n V0n]
            # prologue: the first-chunk working set wins the DMA pipe in exact
            # need-order (q pass deps first), remaining constants trail.
            xth0 = xtp.tile([128, 8, 512], FP8, tag="xth", name="xth0_0")
            xtl0 = xtp.tile([128, 8, 512], FP8, tag="xtl", name="xtl0_0")
            nc.scalar.dma_start(out=wq_h, in_=wqth[:, :, :])
            nc.scalar.dma_start(out=wk_h, in_=wkth[:, :, :])
            nc.scalar.dma_start(out=xth0, in_=xbth[:, :, 0:512])
            nc.scalar.dma_start(out=xtl0, in_=xbtl[:, :, 0:512])
            nc.scalar.dma_start(out=wq_l, in_=wqtl[:, :, :])
            nc.scalar.dma_start(out=wk_l, in_=wktl[:, :, :])
            state[("xt", 0, 0)] = (xth0, xtl0)
            nc.gpsimd.dma_start(out=wv_h, in_=wvth[:, :, :])
            nc.gpsimd.dma_start(out=wv_l, in_=wvtl[:, :, :])
            nc.gpsimd.dma_start(out=psw_sb, in_=pswap[:, :])
            nc.gpsimd.dma_start(out=msq_sb, in_=masksq[:, :])
            nc.sync.dma_start(out=sinp_a, in_=sinp2[:, :, 0:512])
            nc.sync.dma_start(out=cos_a, in_=cos2[:, :, 0:512])
            nc.sync.dma_start(out=sinp_b, in_=sinp2[:, :, 512:S])
            nc.sync.dma_start(out=cos_b, in_=cos2[:, :, 512:S])
            xt_load(0, 1, eng=nc.scalar)
            xt_load(0, 2, eng=nc.sync)
            xt_load(0, 3, eng=nc.sync)
            nc.gpsimd.dma_start(out=wot_sb, in_=wot[:, :])
            nc.scalar.activation(warm, psw_sb[:, 0:2], AF.Exp)

            # merged pipeline: per batch-cycle, interleave attn(b) blocks
            # with proj(b+1) sections and oproj(b-1) pairs so every engine
            # sees a mixed diet continuously. Attention on chunk 0 starts as
            # soon as its projection lands to cover the x-DMA-bound prologue.
            proj_chunk(0, 0)
            scores_block(0, 0)
            proj_chunk(0, 1)
            pv_block(0, 0)
            scores_block(0, 1)
            proj_chunk(0, 2)
            pv_block(0, 1)
            scores_block(0, 2)
            proj_chunk(0, 3)

            def osec(b, ts, alt=False, act_copy=None):
                if b is not None and b >= 0:
                    oproj(b, ts, alt=alt, act_copy=act_copy)

            # steady cycles: cycle b finishes attn(b), runs proj(b+1),
            # starts attn(b+1) through qc2/V1, and drains oproj(b-1)/oproj(b).
            for b in range(nb):
                n = b + 1 if b + 1 < nb else None
                prv = b - 1 if b > 0 else None
                if n is not None:
                    xt_load(n, 0)
                    xt_load(n, 1)
                    scores_block(b, 3)
                    pv_block(b, 2)
                    proj_qk(n, 0)
                    osec(prv, range(12, 16))
                    if prv is not None:
                        release(prv)
                    proj_vr(n, 0)
                    pv_block(b, 3)
                    xt_load(n, 2)
                    proj_qk(n, 1)
                    osec(b, range(0, 2))
                    proj_vr(n, 1)
                    scores_block(n, 0)
                    xt_load(n, 3)
                    proj_qk(n, 2)
                    osec(b, range(2, 6))
                    proj_vr(n, 2)
                    scores_block(n, 1)
                    pv_block(n, 0)
                    proj_qk(n, 3)
                    osec(b, range(6, 10))
                    proj_vr(n, 3)
                    scores_block(n, 2)
                    pv_block(n, 1)
                    osec(b, range(10, 12))
                else:
                    # last batch: spread its own o_proj through its attn;
                    # copies lean on ACT (exp stream dries up here) and the
                    # final pv block is split by query half so its o_proj
                    # tiles pipeline against the remaining pv matmuls.
                    scores_block(b, 3)
                    osec(prv, range(12, 14))
                    osec(b, range(0, 2))
                    osec(prv, range(14, 16), act_copy=True)
                    if prv is not None:
                        release(prv)
                    pv_block(b, 2)
                    osec(b, range(2, 5))
                    osec(b, range(5, 8), act_copy=True)
                    pv3 = psp.tile([128, 1024], F32, tag="pv", bufs=1, name="pv3f")
                    pv_block(b, 3, cols=(0, 256), keep_p=True, pv=pv3)
                    osec(b, range(8, 10))
                    pv_block(b, 3, cols=(256, 512), pv=pv3)
                    oproj(b, range(12, 14), act_copy=True)
                    osec(b, range(10, 12), act_copy=True)
                    oproj(b, range(14, 16), act_copy=True)
                    release(b)

    nc.compile()
    return nc


_NC_CACHE = {}


def _get_nc(nb=B):
    if nb not in _NC_CACHE:
        _NC_CACHE[nb] = build_nc(nb)
    return _NC_CACHE[nb]


def _f8(a):
    return a.astype(F8NP)


def _host_prep(x, Wq, Wk, Wv, Wo):
    x2 = np.ascontiguousarray(x.reshape(BS, D))
    xs = (x2 * SX).astype(np.float32)
    xh = _f8(xs)
    xl = _f8(xs - xh.astype(np.float32))

    def xbt_layout(a):  # [BS, D] fp8 -> [128, 8, BS]
        return np.ascontiguousarray(a.reshape(BS, 8, 128).transpose(2, 1, 0))

    xbth = xbt_layout(xh)
    xbtl = xbt_layout(xl)

    half = 32
    inv_freq = 1.0 / (ROPE_THETA ** (np.arange(half, dtype=np.float64) / half))
    freqs = np.arange(S, dtype=np.float64)[:, None] * inv_freq[None, :]
    c_ = np.cos(freqs).astype(np.float32).T      # [32, S]
    s_ = np.sin(freqs).astype(np.float32).T
    cos1 = np.tile(c_, (4, 1))                        # [128, S]
    sins1 = np.vstack([-s_, s_, -s_, s_])             # [128, S]

    perm = np.zeros(128, dtype=np.int64)
    partner = np.zeros(128, dtype=np.int64)
    for hh in range(2):
        for j in range(64):
            perm[64 * hh + j] = 64 * hh + (2 * j if j < 32 else 2 * (j - 32) + 1)
            partner[64 * hh + j] = 64 * hh + (j + 32) % 64
    pswap = np.zeros((128, 128), dtype=np.float32)
    pswap[partner, np.arange(128)] = 1.0

    sinp1 = sins1[partner] / SCALE                    # u = ps * sinp trick
    cos1 = cos1 / SCALE                               # fold fp8 descale
    cos2 = np.ascontiguousarray(
        np.broadcast_to(cos1[:, None, :], (128, 2, S))).astype(BFNP)
    sinp2 = np.ascontiguousarray(
        np.broadcast_to(sinp1[:, None, :], (128, 2, S))).astype(BFNP)

    # maskw[p, j] = 1 if j >= p else 0, width 512 (cols >=128 all ones);
    # sliced to the exact exp'd range of each diagonal tile
    jj = np.arange(512)
    masksq = (jj[None, :] >= np.arange(128)[:, None]).astype(np.float32).astype(BFNP)

    def w_hilo(Wsl):  # [128 rows, D] (already permuted/sliced) -> hi,lo [128,8,128]
        ws = (Wsl.T * SW).astype(np.float32)          # [D, 128]
        wh = _f8(ws)
        wl = _f8(ws - wh.astype(np.float32))
        def lay(a):
            return np.ascontiguousarray(a.reshape(8, 128, 128).transpose(1, 0, 2))
        return lay(wh), lay(wl)

    in_maps = []
    for c in range(NCORES):
        sl = slice(128 * c, 128 * c + 128)
        wqh, wql = w_hilo(Wq[sl][perm])
        wkh, wkl = w_hilo(Wk[sl][perm])
        wvh, wvl = w_hilo(Wv[sl])
        in_maps.append({
            "xbth": xbth,
            "xbtl": xbtl,
            "wqth": wqh, "wqtl": wql,
            "wkth": wkh, "wktl": wkl,
            "wvth": wvh, "wvtl": wvl,
            "wot": np.ascontiguousarray(Wo[:, sl].T).astype(BFNP),
            "cos2": cos2,
            "sinp2": sinp2,
            "pswap": pswap.astype(BFNP),
            "masksq": masksq,
        })
    return in_maps


def kernel(x, Wq, Wk, Wv, Wo):
    global LAST_RESULTS
    x = np.asarray(x, dtype=np.float32)
    Wq = np.asarray(Wq, dtype=np.float32)
    Wk = np.asarray(Wk, dtype=np.float32)
    Wv = np.asarray(Wv, dtype=np.float32)
    Wo = np.asarray(Wo, dtype=np.float32)

    nc = _get_nc(B)
    in_maps = _host_prep(x, Wq, Wk, Wv, Wo)
    res = run_bass_kernel_spmd(nc, in_maps, core_ids=list(range(NCORES)),
                               trace=TRACE)
    LAST_RESULTS = res
    out = np.zeros((BS, D), dtype=np.float32)
    for c in range(NCORES):
        out += np.asarray(res.results[c]["y"]).astype(np.float32)
    return out.reshape(B, S, D)


# revision 22
# speedup vs baseline: 1.0814x; 1.0027x over previous
"""Causal MHA with RoPE on 8 Trainium2 NeuronCores.

Sharding: tensor-parallel over heads. Core c owns heads {2c, 2c+1} (a 128-wide
slice of the model dim). Each core computes Q/K/V projections for its heads,
full causal attention, and a partial o_proj; the host sums the 8 partial
outputs (the "all-reduce").

v3 (on top of the pipelined v2):
  - Q/K/V projections run in fp8e4 DoubleRow mode (4x PE throughput per
    column): host ships x and W as hi+lo fp8 pairs at a common power-of-2
    scale (sx*sw = 2^13); the 3-term product (xh*Wh + xl*Wh + xh*Wl) restores
    ~bf16 accuracy. The 2^13 descale is folded into the host RoPE tables for
    q/k and into the ones-rows value (denominator trick) for v.
  - softmax reciprocals read the PSUM ones-rows directly with
    partition-offset APs (no staging copies).
  - o_proj PSUM->SBUF copies run on the idle Pool engine; y DMAs issue from
    the SP queue.
  - prologue DMAs ordered so the first projection matmul starts ~3us in.

Device layouts (per core):
  x.T   [128 i-part, 8 i-tile, t]  fp8 hi+lo
  qk_sb [128 hd, 2(q/k), t] bf16;  hd = [head A (ev 0:32, od 32:64), head B]
  scores S.T [k, q] per 128-key tile; P = exp(S.T/8) bf16 in SBUF
  PV: v_sb k-tiles [VA(64) | SCALE-rows(64) | VB(64)]; scale-rows give denoms
  o_proj: ot [128 hd, t] bf16 (stationary) x Wo.T [128 hd, 1024] -> y bf16
"""
import sys
sys.path.insert(0, '/opt/trn_rl_repo')

import numpy as np
import ml_dtypes

import concourse.bass as bass
from concourse import bacc
import concourse.mybir as mybir
import concourse.tile as tile
from concourse.bass_utils import run_bass_kernel_spmd

BFNP = ml_dtypes.bfloat16
F8NP = ml_dtypes.float8_e4m3
F32 = mybir.dt.float32
BF16 = mybir.dt.bfloat16
FP8 = mybir.dt.float8e4
DR = mybir.MatmulPerfMode.DoubleRow
AF = mybir.ActivationFunctionType

B, S, D = 4, 2048, 1024
NCORES = 8
BS = B * S
ROPE_THETA = 10000.0

SX = 16.0          # x fp8 scale
SW = 512.0         # W fp8 scale
SCALE = SX * SW    # folded out via rope tables (q,k) and ones-rows (v)

TRACE = False
LAST_RESULTS = None
PE_LABELS = []
PE_LABEL_BY_NAME = {}


def build_nc(nb=B):
    global PE_LABELS
    PE_LABELS = []
    nc = bacc.Bacc()

    _mm = nc.tensor.matmul
    def _mm_tagged(*a, _label="?", **k):
        PE_LABELS.append(_label)
        r = _mm(*a, **k)
        PE_LABEL_BY_NAME[r.ins.name] = _label
        return r
    nc.tensor.matmul = _mm_tagged
    xbth = nc.dram_tensor("xbth", [128, 8, BS], FP8, kind="ExternalInput")
    xbtl = nc.dram_tensor("xbtl", [128, 8, BS], FP8, kind="ExternalInput")
    # boot: [wq_h|wk_h|xth00|xtl00|wq_l|wk_l|wv_h|wv_l] packed so the first
    # chunk's full working set arrives in 3 ordered DMAs on one queue
    boot = nc.dram_tensor("boot", [128, 14336], FP8, kind="ExternalInput")
    wot = nc.dram_tensor("wot", [128, D], BF16, kind="ExternalInput")
    cos2 = nc.dram_tensor("cos2", [128, 2, S], BF16, kind="ExternalInput")
    sinp2 = nc.dram_tensor("sinp2", [128, 2, S], BF16, kind="ExternalInput")
    pswap = nc.dram_tensor("pswap", [128, 128], BF16, kind="ExternalInput")
    masksq = nc.dram_tensor("masksq", [128, 512], BF16, kind="ExternalInput")
    y = nc.dram_tensor("y", [BS, D], BF16, kind="ExternalOutput")

    with tile.TileContext(nc) as tc:
        with tc.tile_pool(name="const", bufs=1) as constp, \
             tc.tile_pool(name="xt", bufs=4) as xtp, \
             tc.tile_pool(name="qk", bufs=2) as qkp, \
             tc.tile_pool(name="vsb", bufs=2) as vsp, \
             tc.tile_pool(name="u", bufs=4) as up, \
             tc.tile_pool(name="ptile", bufs=34) as pp, \
             tc.tile_pool(name="otp", bufs=2) as otp, \
             tc.tile_pool(name="rc", bufs=4) as rcp, \
             tc.tile_pool(name="yout", bufs=7) as yop, \
             tc.tile_pool(name="psum", bufs=1, space="PSUM") as psp:

            # ---- constant tiles (DMAs emitted in the prologue below) ----
            boot_sb = constp.tile([128, 14336], FP8)

            def bview(o, n, t):  # [128, n*t] slice -> [128, n, t]
                return boot_sb[:, o:o + n * t].rearrange("p (a c) -> p a c", a=n)

            wq_h = bview(0, 8, 128)
            wk_h = bview(1024, 8, 128)
            xth00 = bview(2048, 8, 512)
            xtl00 = bview(6144, 8, 512)
            wq_l = bview(10240, 8, 128)
            wk_l = bview(11264, 8, 128)
            wv_h = bview(12288, 8, 128)
            wv_l = bview(13312, 8, 128)
            sinp_a = constp.tile([128, 2, 512], BF16)
            cos_a = constp.tile([128, 2, 512], BF16)
            sinp_b = constp.tile([128, 2, S - 512], BF16)
            cos_b = constp.tile([128, 2, S - 512], BF16)
            psw_sb = constp.tile([128, 128], BF16)
            msq_sb = constp.tile([128, 512], BF16)
            wot_sb = constp.tile([128, D], BF16)
            warm = constp.tile([128, 2], F32)

            # ---- per-batch state (bufs=2 pools ring across batches) ----
            state = {}

            def xt_load(b, c, eng=None, enl=None):
                tb0 = (b % B) * S + 512 * c
                xth = xtp.tile([128, 8, 512], FP8, tag="xth", name=f"xth{b}_{c}")
                xtl = xtp.tile([128, 8, 512], FP8, tag="xtl", name=f"xtl{b}_{c}")
                (eng or nc.sync).dma_start(out=xth, in_=xbth[:, :, tb0:tb0 + 512])
                (enl or eng or nc.sync).dma_start(out=xtl, in_=xbtl[:, :, tb0:tb0 + 512])
                state[("xt", b, c)] = (xth, xtl)

            def proj_qk(b, c):
                """PE: 24 DoubleRow qk mms. DVE: u/cc rope muls (emitted here
                so they drain the proj-tag psum ring early)."""
                if ("qtr", b) not in state:
                    state[("qtr", b)] = qkp.tile([128, S], BF16, tag="qtr", name=f"qtr{b}")
                    state[("ktr", b)] = qkp.tile([128, S], BF16, tag="ktr", name=f"ktr{b}")
                    v = vsp.tile([128, 16, 192], BF16, tag="v", name=f"v{b}")
                    nc.gpsimd.memset(v[:, :, 64:128], SCALE)
                    state[("v", b)] = v
                t0 = 512 * c
                xth, xtl = state[("xt", b, c)]
                qk_ps = []
                for a, w_h, w_l in ((0, wq_h, wq_l), (1, wk_h, wk_l)):
                    ps = psp.tile([128, 512], F32, tag="proj", bufs=2, name=f"qk{b}_{c}_{a}")
                    nmm = 0
                    for xs, ws in ((xth, w_h), (xtl, w_h), (xth, w_l)):
                        for m in range(4):
                            nc.tensor.matmul(ps, ws[:, 2 * m:2 * m + 2, :],
                                             xs[:, 2 * m:2 * m + 2, :],
                                             start=(nmm == 0), stop=(nmm == 11),
                                             perf_mode=DR,
                                             _label=f"proj{'QK'[a]} b{b} c{c} m{nmm}")
                            nmm += 1
                    qk_ps.append(ps)
                u_sb = up.tile([128, 2, 512], BF16, tag="u")
                cc_sb = up.tile([128, 2, 512], BF16, tag="cc")
                sinp_t = sinp_a if c == 0 else sinp_b[:, :, t0 - 512:t0]
                cos_t = cos_a if c == 0 else cos_b[:, :, t0 - 512:t0]
                for a in range(2):
                    nc.vector.tensor_mul(u_sb[:, a, :], qk_ps[a], sinp_t[:, a, :])
                    nc.vector.tensor_mul(cc_sb[:, a, :], qk_ps[a], cos_t[:, a, :])
                state[("ucc", b, c)] = (u_sb, cc_sb)

            def proj_vr(b, c):
                """PE: 48 DoubleRow v mms + 2 swap mms. DVE: rope adds +
                v copies."""
                v_sb = state[("v", b)]
                qk_dst = (state[("qtr", b)], state[("ktr", b)])
                t0 = 512 * c
                xth, xtl = state.pop(("xt", b, c))
                u_sb, cc_sb = state.pop(("ucc", b, c))
                vt_ps = psp.tile([128, 512], F32, tag="proj", bufs=2)
                for tt in range(4):
                    nmm = 0
                    for xs, ws in ((xth, wv_h), (xtl, wv_h), (xth, wv_l)):
                        for m in range(4):
                            nc.tensor.matmul(vt_ps[:, 128 * tt:128 * tt + 128],
                                             xs[:, 2 * m:2 * m + 2, 128 * tt:128 * tt + 128],
                                             ws[:, 2 * m:2 * m + 2, :],
                                             start=(nmm == 0), stop=(nmm == 11),
                                             perf_mode=DR,
                                             _label=f"projV b{b} c{c} t{tt} m{nmm}")
                            nmm += 1
                for a in range(2):
                    if a == 0:
                        sw_ps = psp.tile([128, 512], F32, tag="pv", bufs=1, name=f"sw{b}_{c}_{a}")
                    else:
                        sw_ps = psp.tile([128, 512], F32, tag="proj", bufs=2, name=f"sw{b}_{c}_{a}")
                    nc.tensor.matmul(sw_ps, psw_sb, u_sb[:, a, :],
                                     start=True, stop=True, _label=f"swap b{b} c{c} a{a}")
                    nc.vector.tensor_add(qk_dst[a][:, t0:t0 + 512], sw_ps, cc_sb[:, a, :])
                # v: [tok%128, tt, hd] -> v_sb ktiles [VA(64) | SCALE | VB(64)]
                vv = vt_ps.rearrange("p (t c) -> p t c", t=4)
                nc.vector.tensor_copy(v_sb[:, 4 * c:4 * c + 4, 0:64], vv[:, :, 0:64])
                nc.vector.tensor_copy(v_sb[:, 4 * c:4 * c + 4, 128:192], vv[:, :, 64:128])

            def proj_chunk(b, c):
                proj_qk(b, c)
                proj_vr(b, c)

            def scores_block(b, qc):
                """PE: scores mms (trimmed on diagonal). ACT: exps. DVE: masks."""
                qtr, ktr = state[("qtr", b)], state[("ktr", b)]
                q0 = 512 * qc
                for kp in range(2 * (qc + 1)):
                    diag = kp >= 2 * qc
                    for hh in range(2):
                        h0 = 64 * hh
                        p_t = pp.tile([128, 1024], BF16, tag="p", name=f"p{b}_{qc}_{kp}_{hh}")
                        st = psp.tile([128, 1024], F32, tag="st", bufs=2, name=f"st{b}_{qc}_{kp}_{hh}")
                        for j in range(2):
                            ki = 2 * kp + j
                            d = ki - 4 * qc
                            trim = 128 * d if diag else 0
                            nc.tensor.matmul(
                                st[:, 512 * j + trim:512 * j + 512],
                                ktr[h0:h0 + 64, 128 * ki:128 * ki + 128],
                                qtr[h0:h0 + 64, q0 + trim:q0 + 512],
                                start=True, stop=True,
                                _label=f"score b{b} q{qc} kp{kp} h{hh} j{j}")
                            if diag:
                                nc.scalar.activation(
                                    p_t[:, 512 * j + trim:512 * j + 512],
                                    st[:, 512 * j + trim:512 * j + 512],
                                    AF.Exp, scale=0.125)
                                # mask only bites in the first 128 cols
                                # (jcol >= p is trivially true beyond)
                                nc.gpsimd.tensor_mul(
                                    p_t[:, 512 * j + trim:512 * j + trim + 128],
                                    p_t[:, 512 * j + trim:512 * j + trim + 128],
                                    msq_sb[:, 0:128])
                        if not diag:
                            nc.scalar.activation(p_t, st, AF.Exp, scale=0.125)
                        state[("p", b, qc, kp, hh)] = p_t

            def pv_block(b, qc, cols=(0, 512), keep_p=False, pv=None):
                """PE: pv mms. DVE: 2 recips + 2 muls -> ot.
                cols selects a query sub-range of the 512-wide block (used to
                pipeline the final block against its o_proj)."""
                if ("ot", b) not in state:
                    state[("ot", b)] = otp.tile([128, S], BF16, tag="ot", name=f"ot{b}")
                ot = state[("ot", b)]
                v_sb = state[("v", b)]
                c0, c1 = cols
                q0 = 512 * qc
                nk = 4 * qc + 4
                if pv is None:
                    pv = psp.tile([128, 1024], F32, tag="pv", bufs=1, name=f"pv{b}_{qc}_{c0}")
                for hh in range(2):
                    col0 = 0 if hh == 0 else 64
                    kis = [ki for ki in range(nk)
                           if max(128 * (ki - 4 * qc), c0) < c1]
                    for ii, ki in enumerate(kis):
                        d = ki - 4 * qc
                        trim = max(128 * d if d >= 0 else 0, c0)
                        kp, j = divmod(ki, 2)
                        p_t = state[("p", b, qc, kp, hh)]
                        nc.tensor.matmul(
                            pv[:, 512 * hh + trim:512 * hh + c1],
                            v_sb[:, ki, col0:col0 + 128],
                            p_t[:, 512 * j + trim:512 * j + c1],
                            start=(ii == 0), stop=(ii == len(kis) - 1),
                            _label=f"pv b{b} q{qc} h{hh} ki{ki}")
                if not keep_p:
                    for kp in range(2 * (qc + 1)):
                        for hh in range(2):
                            state.pop(("p", b, qc, kp, hh))
                # denominators sit replicated in the ones-rows:
                #   hh=0: rows 64:128 ; hh=1: rows 0:64
                r2 = rcp.tile([128, 512], F32, tag="r2")
                nc.vector.reciprocal(r2[0:64, c0:c1], pv[64:128, c0:c1])
                nc.vector.reciprocal(r2[64:128, c0:c1], pv[0:64, 512 + c0:512 + c1])
                nc.vector.tensor_mul(ot[0:64, q0 + c0:q0 + c1], pv[0:64, c0:c1],
                                     r2[0:64, c0:c1])
                nc.vector.tensor_mul(ot[64:128, q0 + c0:q0 + c1],
                                     pv[64:128, 512 + c0:512 + c1],
                                     r2[64:128, c0:c1])

            def oproj(b, tts, alt=False, act_copy=None):
                """PE: 2 mms per t-tile into one wide tile; mostly-DVE wide
                copies (ACT every 4th by default). DMA y on SP queue."""
                ot = state[("ot", b)]
                tb0 = (b % B) * S
                for tt in tts:
                    yo = yop.tile([128, 1024], BF16, tag="yo")
                    if alt and tt % 2 == 1:
                        op_ps = psp.tile([128, 1024], F32, tag="pv", bufs=1, name=f"op{b}_{tt}")
                    else:
                        op_ps = psp.tile([128, 1024], F32, tag="st", bufs=2, name=f"op{b}_{tt}")
                    for oc in range(2):
                        nc.tensor.matmul(op_ps[:, 512 * oc:512 * oc + 512],
                                         ot[:, 128 * tt:128 * tt + 128],
                                         wot_sb[:, 512 * oc:512 * oc + 512],
                                         start=True, stop=True,
                                         _label=f"oproj b{b} t{tt} o{oc}")
                    if act_copy == 'mix':
                        on_act = (tt % 2 == 0)
                    else:
                        on_act = (tt % 4 == 0) if act_copy is None else act_copy
                    if on_act:
                        nc.scalar.activation(yo, op_ps, AF.Copy)
                    else:
                        nc.vector.tensor_copy(yo, op_ps)
                    nc.sync.dma_start(out=y[tb0 + 128 * tt:tb0 + 128 * tt + 128, :], in_=yo)

            def release(b):
                state.pop(("qtr", b))
                state.pop(("ktr", b))
                state.pop(("v", b))
                state.pop(("ot", b))

            # ---- pipelined emission ----
            # steady state per batch n (prev = n-1):
            #  [S2 V1] [P0n S3 V2] [V3 P1n] [O P2n] [P3n S0n] [S1n V0n]
            # prologue: the first-chunk working set (boot) wins the DMA pipe
            # in exact need-order on one queue; tables trail on gpsimd/sync.
            nc.scalar.dma_start(out=boot_sb[:, 0:6144], in_=boot[:, 0:6144])
            nc.scalar.dma_start(out=boot_sb[:, 6144:10240], in_=boot[:, 6144:10240])
            nc.scalar.dma_start(out=boot_sb[:, 10240:14336], in_=boot[:, 10240:14336])
            state[("xt", 0, 0)] = (xth00, xtl00)
            nc.gpsimd.dma_start(out=sinp_a, in_=sinp2[:, :, 0:512])
            nc.gpsimd.dma_start(out=cos_a, in_=cos2[:, :, 0:512])
            nc.gpsimd.dma_start(out=psw_sb, in_=pswap[:, :])
            nc.gpsimd.dma_start(out=msq_sb, in_=masksq[:, :])
            nc.gpsimd.dma_start(out=sinp_b, in_=sinp2[:, :, 512:S])
            nc.gpsimd.dma_start(out=cos_b, in_=cos2[:, :, 512:S])
            xt_load(0, 1, eng=nc.scalar)
            xt_load(0, 2, eng=nc.sync)
            xt_load(0, 3, eng=nc.sync)
            nc.gpsimd.dma_start(out=wot_sb, in_=wot[:, :])
            nc.scalar.activation(warm, psw_sb[:, 0:2], AF.Exp)

            # merged pipeline: per batch-cycle, interleave attn(b) blocks
            # with proj(b+1) sections and oproj(b-1) pairs so every engine
            # sees a mixed diet continuously. Attention on chunk 0 starts as
            # soon as its projection lands to cover the x-DMA-bound prologue.
            proj_chunk(0, 0)
            scores_block(0, 0)
            proj_chunk(0, 1)
            pv_block(0, 0)
            scores_block(0, 1)
            proj_chunk(0, 2)
            pv_block(0, 1)
            scores_block(0, 2)
            proj_chunk(0, 3)

            def osec(b, ts, alt=False, act_copy=None):
                if b is not None and b >= 0:
                    oproj(b, ts, alt=alt, act_copy=act_copy)

            # steady cycles: cycle b finishes attn(b), runs proj(b+1),
            # starts attn(b+1) through qc2/V1, and drains oproj(b-1)/oproj(b).
            for b in range(nb):
                n = b + 1 if b + 1 < nb else None
                prv = b - 1 if b > 0 else None
                if n is not None:
                    xt_load(n, 0)
                    xt_load(n, 1)
                    scores_block(b, 3)
                    pv_block(b, 2)
                    proj_qk(n, 0)
                    osec(prv, range(12, 16))
                    if prv is not None:
                        release(prv)
                    proj_vr(n, 0)
                    pv_block(b, 3)
                    xt_load(n, 2)
                    proj_qk(n, 1)
                    osec(b, range(0, 2))
                    proj_vr(n, 1)
                    scores_block(n, 0)
                    xt_load(n, 3)
                    proj_qk(n, 2)
                    osec(b, range(2, 6))
                    proj_vr(n, 2)
                    scores_block(n, 1)
                    pv_block(n, 0)
                    proj_qk(n, 3)
                    osec(b, range(6, 10))
                    proj_vr(n, 3)
                    scores_block(n, 2)
                    pv_block(n, 1)
                    osec(b, range(10, 12))
                else:
                    # last batch: spread its own o_proj through its attn;
                    # copies lean on ACT (exp stream dries up here) and the
                    # final pv block is split by query half so its o_proj
                    # tiles pipeline against the remaining pv matmuls.
                    scores_block(b, 3)
                    osec(prv, range(12, 14))
                    osec(b, range(0, 2))
                    osec(prv, range(14, 16), act_copy='mix')
                    if prv is not None:
                        release(prv)
                    pv_block(b, 2)
                    osec(b, range(2, 5))
                    osec(b, range(5, 8), act_copy='mix')
                    pv3 = psp.tile([128, 1024], F32, tag="pv", bufs=1, name="pv3f")
                    pv_block(b, 3, cols=(0, 256), keep_p=True, pv=pv3)
                    osec(b, range(8, 10), act_copy='mix')
                    pv_block(b, 3, cols=(256, 512), pv=pv3)
                    oproj(b, range(12, 14), act_copy='mix')
                    osec(b, range(10, 12), act_copy='mix')
                    oproj(b, range(14, 16), act_copy='mix')
                    release(b)

    nc.compile()
    return nc


_NC_CACHE = {}


def _get_nc(nb=B):
    if nb not in _NC_CACHE:
        _NC_CACHE[nb] = build_nc(nb)
    return _NC_CACHE[nb]


def _f8(a):
    return a.astype(F8NP)


def _host_prep(x, Wq, Wk, Wv, Wo):
    x2 = np.ascontiguousarray(x.reshape(BS, D))
    xs = (x2 * SX).astype(np.float32)
    xh = _f8(xs)
    xl = _f8(xs - xh.astype(np.float32))

    def xbt_layout(a):  # [BS, D] fp8 -> [128, 8, BS]
        return np.ascontiguousarray(a.reshape(BS, 8, 128).transpose(2, 1, 0))

    xbth = xbt_layout(xh)
    xbtl = xbt_layout(xl)

    half = 32
    inv_freq = 1.0 / (ROPE_THETA ** (np.arange(half, dtype=np.float64) / half))
    freqs = np.arange(S, dtype=np.float64)[:, None] * inv_freq[None, :]
    c_ = np.cos(freqs).astype(np.float32).T      # [32, S]
    s_ = np.sin(freqs).astype(np.float32).T
    cos1 = np.tile(c_, (4, 1))                        # [128, S]
    sins1 = np.vstack([-s_, s_, -s_, s_])             # [128, S]

    perm = np.zeros(128, dtype=np.int64)
    partner = np.zeros(128, dtype=np.int64)
    for hh in range(2):
        for j in range(64):
            perm[64 * hh + j] = 64 * hh + (2 * j if j < 32 else 2 * (j - 32) + 1)
            partner[64 * hh + j] = 64 * hh + (j + 32) % 64
    pswap = np.zeros((128, 128), dtype=np.float32)
    pswap[partner, np.arange(128)] = 1.0

    sinp1 = sins1[partner] / SCALE                    # u = ps * sinp trick
    cos1 = cos1 / SCALE                               # fold fp8 descale
    cos2 = np.ascontiguousarray(
        np.broadcast_to(cos1[:, None, :], (128, 2, S))).astype(BFNP)
    sinp2 = np.ascontiguousarray(
        np.broadcast_to(sinp1[:, None, :], (128, 2, S))).astype(BFNP)

    # maskw[p, j] = 1 if j >= p else 0, width 512 (cols >=128 all ones);
    # sliced to the exact exp'd range of each diagonal tile
    jj = np.arange(512)
    masksq = (jj[None, :] >= np.arange(128)[:, None]).astype(np.float32).astype(BFNP)

    def w_hilo(Wsl):  # [128 rows, D] (already permuted/sliced) -> hi,lo [128,8,128]
        ws = (Wsl.T * SW).astype(np.float32)          # [D, 128]
        wh = _f8(ws)
        wl = _f8(ws - wh.astype(np.float32))
        def lay(a):
            return np.ascontiguousarray(a.reshape(8, 128, 128).transpose(1, 0, 2))
        return lay(wh), lay(wl)

    in_maps = []
    for c in range(NCORES):
        sl = slice(128 * c, 128 * c + 128)
        wqh, wql = w_hilo(Wq[sl][perm])
        wkh, wkl = w_hilo(Wk[sl][perm])
        wvh, wvl = w_hilo(Wv[sl])

        def flat(a):
            return np.ascontiguousarray(a).reshape(128, -1)

        boot = np.concatenate(
            [flat(wqh), flat(wkh), flat(xbth[:, :, 0:512]), flat(xbtl[:, :, 0:512]),
             flat(wql), flat(wkl), flat(wvh), flat(wvl)], axis=1)
        in_maps.append({
            "xbth": xbth,
            "xbtl": xbtl,
            "boot": boot,
            "wot": np.ascontiguousarray(Wo[:, sl].T).astype(BFNP),
            "cos2": cos2,
            "sinp2": sinp2,
            "pswap": pswap.astype(BFNP),
            "masksq": masksq,
        })
    return in_maps


def kernel(x, Wq, Wk, Wv, Wo):
    global LAST_RESULTS
    x = np.asarray(x, dtype=np.float32)
    Wq = np.asarray(Wq, dtype=np.float32)
    Wk = np.asarray(Wk, dtype=np.float32)
    Wv = np.asarray(Wv, dtype=np.float32)
    Wo = np.asarray(Wo, dtype=np.float32)

    nc = _get_nc(B)
    in_maps = _host_prep(x, Wq, Wk, Wv, Wo)
    res = run_bass_kernel_spmd(nc, in_maps, core_ids=list(range(NCORES)),
                               trace=TRACE)
    LAST_RESULTS = res
    out = np.zeros((BS, D), dtype=np.float32)
    for c in range(NCORES):
        out += np.asarray(res.results[c]["y"]).astype(np.float32)
    return out.reshape(B, S, D)


# revision 39
# speedup vs baseline: 1.0949x; 1.0125x over previous
"""Causal MHA with RoPE on 8 Trainium2 NeuronCores.

Sharding: tensor-parallel over heads. Core c owns heads {2c, 2c+1} (a 128-wide
slice of the model dim). Each core computes Q/K/V projections for its heads,
full causal attention, and a partial o_proj; the host sums the 8 partial
outputs (the "all-reduce").

v3 (on top of the pipelined v2):
  - Q/K/V projections run in fp8e4 DoubleRow mode (4x PE throughput per
    column): host ships x and W as hi+lo fp8 pairs at a common power-of-2
    scale (sx*sw = 2^13); the 3-term product (xh*Wh + xl*Wh + xh*Wl) restores
    ~bf16 accuracy. The 2^13 descale is folded into the host RoPE tables for
    q/k and into the ones-rows value (denominator trick) for v.
  - softmax reciprocals read the PSUM ones-rows directly with
    partition-offset APs (no staging copies).
  - o_proj PSUM->SBUF copies run on the idle Pool engine; y DMAs issue from
    the SP queue.
  - prologue DMAs ordered so the first projection matmul starts ~3us in.

Device layouts (per core):
  x.T   [128 i-part, 8 i-tile, t]  fp8 hi+lo
  qk_sb [128 hd, 2(q/k), t] bf16;  hd = [head A (ev 0:32, od 32:64), head B]
  scores S.T [k, q] per 128-key tile; P = exp(S.T/8) bf16 in SBUF
  PV: v_sb k-tiles [VA(64) | SCALE-rows(64) | VB(64)]; scale-rows give denoms
  o_proj: ot [128 hd, t] bf16 (stationary) x Wo.T [128 hd, 1024] -> y bf16
"""
import sys
sys.path.insert(0, '/opt/trn_rl_repo')

import numpy as np
import ml_dtypes

import concourse.bass as bass
from concourse import bacc
import concourse.mybir as mybir
import concourse.tile as tile
from concourse.bass_utils import run_bass_kernel_spmd

BFNP = ml_dtypes.bfloat16
F8NP = ml_dtypes.float8_e4m3
F32 = mybir.dt.float32
BF16 = mybir.dt.bfloat16
FP8 = mybir.dt.float8e4
DR = mybir.MatmulPerfMode.DoubleRow
AF = mybir.ActivationFunctionType

B, S, D = 4, 2048, 1024
NCORES = 8
BS = B * S
ROPE_THETA = 10000.0

SX = 16.0          # x fp8 scale
SW = 512.0         # W fp8 scale
SCALE = SX * SW    # folded out via rope tables (q,k) and ones-rows (v)

TRACE = False
# queue per prologue DMA: sinp_a,cos_a,psw,msq,sinp_b,cos_b,
#                         xt01h,xt01l,xt02h,xt02l,xt03h,xt03l,wot
# a=ACT  s=SP(sync)  g=gpsimd(Pool)
PROLOGUE_QUEUES = "ggggggaaggssg"
# weave ratios (score_per_round, other_per_round) per phase
WR = {'p1': (2, 2), 'p2': (2, 1), 'c1': (3, 2), 'c2': (1, 3), 'c3': (1, 3),
      'c4': (1, 1), 'l1': (3, 2)}
LAST_RESULTS = None
PE_LABELS = []
PE_LABEL_BY_NAME = {}


def build_nc(nb=B):
    global PE_LABELS
    PE_LABELS = []
    nc = bacc.Bacc()

    _mm = nc.tensor.matmul
    def _mm_tagged(*a, _label="?", **k):
        PE_LABELS.append(_label)
        r = _mm(*a, **k)
        PE_LABEL_BY_NAME[r.ins.name] = _label
        return r
    nc.tensor.matmul = _mm_tagged
    xbth = nc.dram_tensor("xbth", [128, 8, BS], FP8, kind="ExternalInput")
    xbtl = nc.dram_tensor("xbtl", [128, 8, BS], FP8, kind="ExternalInput")
    # boot: [wq_h|wk_h|xth00|xtl00|wq_l|wk_l|wv_h|wv_l] packed so the first
    # chunk's full working set arrives in 3 ordered DMAs on one queue
    boot = nc.dram_tensor("boot", [128, 14336], FP8, kind="ExternalInput")
    wot = nc.dram_tensor("wot", [128, D], BF16, kind="ExternalInput")
    cos2 = nc.dram_tensor("cos2", [128, 2, S], BF16, kind="ExternalInput")
    sinp2 = nc.dram_tensor("sinp2", [128, 2, S], BF16, kind="ExternalInput")
    pswap = nc.dram_tensor("pswap", [128, 128], BF16, kind="ExternalInput")
    masksq = nc.dram_tensor("masksq", [128, 512], BF16, kind="ExternalInput")
    y = nc.dram_tensor("y", [BS, D], BF16, kind="ExternalOutput")

    with tile.TileContext(nc) as tc:
        with tc.tile_pool(name="const", bufs=1) as constp, \
             tc.tile_pool(name="xt", bufs=4) as xtp, \
             tc.tile_pool(name="qk", bufs=2) as qkp, \
             tc.tile_pool(name="vsb", bufs=2) as vsp, \
             tc.tile_pool(name="u", bufs=4) as up, \
             tc.tile_pool(name="ptile", bufs=34) as pp, \
             tc.tile_pool(name="otp", bufs=2) as otp, \
             tc.tile_pool(name="rc", bufs=4) as rcp, \
             tc.tile_pool(name="yout", bufs=7) as yop, \
             tc.tile_pool(name="psum", bufs=1, space="PSUM") as psp:

            # ---- constant tiles (DMAs emitted in the prologue below) ----
            boot_sb = constp.tile([128, 14336], FP8)

            def bview(o, n, t):  # [128, n*t] slice -> [128, n, t]
                return boot_sb[:, o:o + n * t].rearrange("p (a c) -> p a c", a=n)

            wq_h = bview(0, 8, 128)
            wk_h = bview(1024, 8, 128)
            xth00 = bview(2048, 8, 512)
            xtl00 = bview(6144, 8, 512)
            wq_l = bview(10240, 8, 128)
            wk_l = bview(11264, 8, 128)
            wv_h = bview(12288, 8, 128)
            wv_l = bview(13312, 8, 128)
            sinp_a = constp.tile([128, 2, 512], BF16)
            cos_a = constp.tile([128, 2, 512], BF16)
            sinp_b = constp.tile([128, 2, S - 512], BF16)
            cos_b = constp.tile([128, 2, S - 512], BF16)
            psw_sb = constp.tile([128, 128], BF16)
            msq_sb = constp.tile([128, 512], BF16)
            wot_sb = constp.tile([128, D], BF16)
            warm = constp.tile([128, 2], F32)

            # ---- per-batch state (bufs=2 pools ring across batches) ----
            state = {}

            def xt_load(b, c, eng=None, enl=None):
                tb0 = (b % B) * S + 512 * c
                xth = xtp.tile([128, 8, 512], FP8, tag="xth", name=f"xth{b}_{c}")
                xtl = xtp.tile([128, 8, 512], FP8, tag="xtl", name=f"xtl{b}_{c}")
                (eng or nc.sync).dma_start(out=xth, in_=xbth[:, :, tb0:tb0 + 512])
                (enl or eng or nc.sync).dma_start(out=xtl, in_=xbtl[:, :, tb0:tb0 + 512])
                state[("xt", b, c)] = (xth, xtl)

            def projqk_a(b, c, a):
                """PE: 12 DoubleRow mms for q or k; DVE u/cc muls after a=1."""
                if ("qtr", b) not in state:
                    state[("qtr", b)] = qkp.tile([128, S], BF16, tag="qtr", name=f"qtr{b}")
                    state[("ktr", b)] = qkp.tile([128, S], BF16, tag="ktr", name=f"ktr{b}")
                    v = vsp.tile([128, 16, 192], BF16, tag="v", name=f"v{b}")
                    nc.gpsimd.memset(v[:, :, 64:128], SCALE)
                    state[("v", b)] = v
                t0 = 512 * c
                xth, xtl = state[("xt", b, c)]
                w_h, w_l = (wq_h, wq_l) if a == 0 else (wk_h, wk_l)
                ps = psp.tile([128, 512], F32, tag="proj", bufs=2, name=f"qk{b}_{c}_{a}")
                nmm = 0
                for xs, ws in ((xth, w_h), (xtl, w_h), (xth, w_l)):
                    for m in range(4):
                        nc.tensor.matmul(ps, ws[:, 2 * m:2 * m + 2, :],
                                         xs[:, 2 * m:2 * m + 2, :],
                                         start=(nmm == 0), stop=(nmm == 11),
                                         perf_mode=DR,
                                         _label=f"proj{'QK'[a]} b{b} c{c} m{nmm}")
                        nmm += 1
                state[("qkps", b, c, a)] = ps
                if a == 1:
                    u_sb = up.tile([128, 2, 512], BF16, tag="u")
                    cc_sb = up.tile([128, 2, 512], BF16, tag="cc")
                    sinp_t = sinp_a if c == 0 else sinp_b[:, :, t0 - 512:t0]
                    cos_t = cos_a if c == 0 else cos_b[:, :, t0 - 512:t0]
                    for aa in range(2):
                        qk_ps = state.pop(("qkps", b, c, aa))
                        nc.vector.tensor_mul(u_sb[:, aa, :], qk_ps, sinp_t[:, aa, :])
                        nc.vector.tensor_mul(cc_sb[:, aa, :], qk_ps, cos_t[:, aa, :])
                    state[("ucc", b, c)] = (u_sb, cc_sb)

            def projqk_steps(b, c):
                return [lambda: projqk_a(b, c, 0), lambda: projqk_a(b, c, 1)]

            def proj_qk(b, c):
                for s in projqk_steps(b, c):
                    s()

            def projvr_vt(b, c, tts):
                """PE: DoubleRow v mms for t-tiles tts."""
                xth, xtl = state[("xt", b, c)]
                if ("vtps", b, c) not in state:
                    state[("vtps", b, c)] = psp.tile([128, 512], F32, tag="proj",
                                                     bufs=2, name=f"vt{b}_{c}")
                vt_ps = state[("vtps", b, c)]
                for tt in tts:
                    nmm = 0
                    for xs, ws in ((xth, wv_h), (xtl, wv_h), (xth, wv_l)):
                        for m in range(4):
                            nc.tensor.matmul(vt_ps[:, 128 * tt:128 * tt + 128],
                                             xs[:, 2 * m:2 * m + 2, 128 * tt:128 * tt + 128],
                                             ws[:, 2 * m:2 * m + 2, :],
                                             start=(nmm == 0), stop=(nmm == 11),
                                             perf_mode=DR,
                                             _label=f"projV b{b} c{c} t{tt} m{nmm}")
                            nmm += 1

            def projvr_fin(b, c):
                """PE: 2 swap mms. DVE: rope adds + v copies."""
                v_sb = state[("v", b)]
                qk_dst = (state[("qtr", b)], state[("ktr", b)])
                t0 = 512 * c
                state.pop(("xt", b, c))
                u_sb, cc_sb = state.pop(("ucc", b, c))
                vt_ps = state.pop(("vtps", b, c))
                for a in range(2):
                    if a == 0:
                        sw_ps = psp.tile([128, 512], F32, tag="pv", bufs=1, name=f"sw{b}_{c}_{a}")
                    else:
                        sw_ps = psp.tile([128, 512], F32, tag="proj", bufs=2, name=f"sw{b}_{c}_{a}")
                    nc.tensor.matmul(sw_ps, psw_sb, u_sb[:, a, :],
                                     start=True, stop=True, _label=f"swap b{b} c{c} a{a}")
                    nc.vector.tensor_add(qk_dst[a][:, t0:t0 + 512], sw_ps, cc_sb[:, a, :])
                # v: [tok%128, tt, hd] -> v_sb ktiles [VA(64) | SCALE | VB(64)]
                vv = vt_ps.rearrange("p (t c) -> p t c", t=4)
                nc.vector.tensor_copy(v_sb[:, 4 * c:4 * c + 4, 0:64], vv[:, :, 0:64])
                nc.vector.tensor_copy(v_sb[:, 4 * c:4 * c + 4, 128:192], vv[:, :, 64:128])

            def projvr_steps(b, c):
                return [lambda: projvr_vt(b, c, (0, 1)),
                        lambda: projvr_vt(b, c, (2, 3)),
                        lambda: projvr_fin(b, c)]

            def proj_vr(b, c):
                for s in projvr_steps(b, c):
                    s()

            def proj_chunk(b, c):
                proj_qk(b, c)
                proj_vr(b, c)

            def weave(*pairs):
                """pairs of (steps, n_per_round): round-robin emission."""
                lists = [list(s) for s, _ in pairs]
                counts = [n for _, n in pairs]
                while any(lists):
                    for li, n in zip(lists, counts):
                        for _ in range(n):
                            if li:
                                li.pop(0)()

            def score_tile(b, qc, kp, hh):
                """One (kp, hh) tile: PE 2 mms -> ACT exp(s) -> Pool mask."""
                qtr, ktr = state[("qtr", b)], state[("ktr", b)]
                q0 = 512 * qc
                diag = kp >= 2 * qc
                h0 = 64 * hh
                p_t = pp.tile([128, 1024], BF16, tag="p", name=f"p{b}_{qc}_{kp}_{hh}")
                st = psp.tile([128, 1024], F32, tag="st", bufs=2, name=f"st{b}_{qc}_{kp}_{hh}")
                for j in range(2):
                    ki = 2 * kp + j
                    d = ki - 4 * qc
                    trim = 128 * d if diag else 0
                    nc.tensor.matmul(
                        st[:, 512 * j + trim:512 * j + 512],
                        ktr[h0:h0 + 64, 128 * ki:128 * ki + 128],
                        qtr[h0:h0 + 64, q0 + trim:q0 + 512],
                        start=True, stop=True,
                        _label=f"score b{b} q{qc} kp{kp} h{hh} j{j}")
                    if diag:
                        nc.scalar.activation(
                            p_t[:, 512 * j + trim:512 * j + 512],
                            st[:, 512 * j + trim:512 * j + 512],
                            AF.Exp, scale=0.125)
                        # mask only bites in the first 128 cols
                        # (jcol >= p is trivially true beyond)
                        nc.gpsimd.tensor_mul(
                            p_t[:, 512 * j + trim:512 * j + trim + 128],
                            p_t[:, 512 * j + trim:512 * j + trim + 128],
                            msq_sb[:, 0:128])
                if not diag:
                    nc.scalar.activation(p_t, st, AF.Exp, scale=0.125)
                state[("p", b, qc, kp, hh)] = p_t

            def scores_steps(b, qc):
                return [(lambda kp=kp, hh=hh: score_tile(b, qc, kp, hh))
                        for kp in range(2 * (qc + 1)) for hh in range(2)]

            def scores_block(b, qc):
                """PE: scores mms (trimmed on diagonal). ACT: exps. Pool: masks."""
                for s in scores_steps(b, qc):
                    s()

            def pv_hh(b, qc, hh, cols, pvkey):
                """PE: one head's pv accumulation group for query cols."""
                if ("ot", b) not in state:
                    state[("ot", b)] = otp.tile([128, S], BF16, tag="ot", name=f"ot{b}")
                v_sb = state[("v", b)]
                c0, c1 = cols
                nk = 4 * qc + 4
                if pvkey not in state:
                    state[pvkey] = psp.tile([128, 1024], F32, tag="pv", bufs=1,
                                            name=f"pv{b}_{qc}_{c0}")
                pv = state[pvkey]
                col0 = 0 if hh == 0 else 64
                kis = [ki for ki in range(nk)
                       if max(128 * (ki - 4 * qc), c0) < c1]
                for ii, ki in enumerate(kis):
                    d = ki - 4 * qc
                    trim = max(128 * d if d >= 0 else 0, c0)
                    kp, j = divmod(ki, 2)
                    p_t = state[("p", b, qc, kp, hh)]
                    nc.tensor.matmul(
                        pv[:, 512 * hh + trim:512 * hh + c1],
                        v_sb[:, ki, col0:col0 + 128],
                        p_t[:, 512 * j + trim:512 * j + c1],
                        start=(ii == 0), stop=(ii == len(kis) - 1),
                        _label=f"pv b{b} q{qc} h{hh} ki{ki}")

            def pv_norm(b, qc, cols, pvkey, keep_p, pop_pv=True):
                """DVE: 2 recips + 2 muls -> ot (denominators sit replicated
                in the ones-rows: hh=0 rows 64:128, hh=1 rows 0:64)."""
                ot = state[("ot", b)]
                c0, c1 = cols
                q0 = 512 * qc
                pv = state.pop(pvkey) if pop_pv else state[pvkey]
                if not keep_p:
                    for kp in range(2 * (qc + 1)):
                        for hh in range(2):
                            state.pop(("p", b, qc, kp, hh))
                r2 = rcp.tile([128, 512], F32, tag="r2")
                nc.vector.reciprocal(r2[0:64, c0:c1], pv[64:128, c0:c1])
                nc.vector.reciprocal(r2[64:128, c0:c1], pv[0:64, 512 + c0:512 + c1])
                nc.vector.tensor_mul(ot[0:64, q0 + c0:q0 + c1], pv[0:64, c0:c1],
                                     r2[0:64, c0:c1])
                nc.vector.tensor_mul(ot[64:128, q0 + c0:q0 + c1],
                                     pv[64:128, 512 + c0:512 + c1],
                                     r2[64:128, c0:c1])

            def pv_steps(b, qc, cols=(0, 512), keep_p=False, pvkey=None, pop_pv=True):
                if pvkey is None:
                    pvkey = ("pvps", b, qc)
                return [lambda: pv_hh(b, qc, 0, cols, pvkey),
                        lambda: pv_hh(b, qc, 1, cols, pvkey),
                        lambda: pv_norm(b, qc, cols, pvkey, keep_p, pop_pv)]

            def pv_block(b, qc, cols=(0, 512), keep_p=False, pvkey=None, pop_pv=True):
                for s in pv_steps(b, qc, cols, keep_p, pvkey, pop_pv):
                    s()

            def oproj_tile(b, tt, alt=False, act_copy=None):
                """PE: 2 mms into one wide tile; ACT-or-DVE wide copy;
                y DMA on SP queue."""
                ot = state[("ot", b)]
                tb0 = (b % B) * S
                yo = yop.tile([128, 1024], BF16, tag="yo")
                if alt and tt % 2 == 1:
                    op_ps = psp.tile([128, 1024], F32, tag="pv", bufs=1, name=f"op{b}_{tt}")
                else:
                    op_ps = psp.tile([128, 1024], F32, tag="st", bufs=2, name=f"op{b}_{tt}")
                for oc in range(2):
                    nc.tensor.matmul(op_ps[:, 512 * oc:512 * oc + 512],
                                     ot[:, 128 * tt:128 * tt + 128],
                                     wot_sb[:, 512 * oc:512 * oc + 512],
                                     start=True, stop=True,
                                     _label=f"oproj b{b} t{tt} o{oc}")
                if act_copy == 'mix':
                    on_act = (tt % 2 == 0)
                else:
                    on_act = (tt % 4 == 0) if act_copy is None else act_copy
                if on_act:
                    nc.scalar.activation(yo, op_ps, AF.Copy)
                else:
                    nc.vector.tensor_copy(yo, op_ps)
                nc.sync.dma_start(out=y[tb0 + 128 * tt:tb0 + 128 * tt + 128, :], in_=yo)

            def oproj_steps(b, tts, alt=False, act_copy=None):
                if b is None or b < 0:
                    return []
                return [(lambda tt=tt: oproj_tile(b, tt, alt, act_copy))
                        for tt in tts]

            def oproj(b, tts, alt=False, act_copy=None):
                for s in oproj_steps(b, tts, alt, act_copy):
                    s()

            def release(b):
                state.pop(("qtr", b))
                state.pop(("ktr", b))
                state.pop(("v", b))
                state.pop(("ot", b))

            # ---- pipelined emission ----
            # steady state per batch n (prev = n-1):
            #  [S2 V1] [P0n S3 V2] [V3 P1n] [O P2n] [P3n S0n] [S1n V0n]
            # prologue: the first-chunk working set (boot) wins the DMA pipe
            # in exact need-order on one queue; tables trail on gpsimd/sync.
            nc.scalar.dma_start(out=boot_sb[:, 0:6144], in_=boot[:, 0:6144])
            nc.scalar.dma_start(out=boot_sb[:, 6144:10240], in_=boot[:, 6144:10240])
            nc.scalar.dma_start(out=boot_sb[:, 10240:14336], in_=boot[:, 10240:14336])
            state[("xt", 0, 0)] = (xth00, xtl00)
            qmap = {'a': nc.scalar, 's': nc.sync, 'g': nc.gpsimd}
            pq = [qmap[ch] for ch in PROLOGUE_QUEUES]
            pq[0].dma_start(out=sinp_a, in_=sinp2[:, :, 0:512])
            pq[1].dma_start(out=cos_a, in_=cos2[:, :, 0:512])
            pq[2].dma_start(out=psw_sb, in_=pswap[:, :])
            pq[3].dma_start(out=msq_sb, in_=masksq[:, :])
            pq[4].dma_start(out=sinp_b, in_=sinp2[:, :, 512:S])
            pq[5].dma_start(out=cos_b, in_=cos2[:, :, 512:S])
            xt_load(0, 1, eng=pq[6], enl=pq[7])
            xt_load(0, 2, eng=pq[8], enl=pq[9])
            xt_load(0, 3, eng=pq[10], enl=pq[11])
            pq[12].dma_start(out=wot_sb, in_=wot[:, :])
            nc.scalar.activation(warm, psw_sb[:, 0:2], AF.Exp)

            # merged pipeline: per batch-cycle, interleave attn(b) blocks
            # with proj(b+1) sections and oproj(b-1) pairs so every engine
            # sees a mixed diet continuously. Attention on chunk 0 starts as
            # soon as its projection lands to cover the x-DMA-bound prologue.
            proj_chunk(0, 0)
            weave((scores_steps(0, 0), 1),
                  (projqk_steps(0, 1) + projvr_steps(0, 1), 1))
            weave((scores_steps(0, 1), WR['p1'][0]),
                  (pv_steps(0, 0) + projqk_steps(0, 2) + projvr_steps(0, 2), WR['p1'][1]))
            weave((scores_steps(0, 2), WR['p2'][0]),
                  (pv_steps(0, 1) + projqk_steps(0, 3) + projvr_steps(0, 3), WR['p2'][1]))

            # steady cycles: cycle b finishes attn(b), runs proj(b+1),
            # starts attn(b+1) through qc2/V1, and drains oproj(b-1)/oproj(b).
            # Score tiles (ACT-heavy) are woven between PE-heavy steps so the
            # exp stream pipelines behind matmuls instead of stalling the
            # 2-slot st psum ring.
            for b in range(nb):
                n = b + 1 if b + 1 < nb else None
                prv = b - 1 if b > 0 else None
                if n is not None:
                    xt_load(n, 0)
                    xt_load(n, 1)
                    weave((scores_steps(b, 3), WR['c1'][0]),
                          (pv_steps(b, 2) + projqk_steps(n, 0)
                           + oproj_steps(prv, range(12, 16))
                           + projvr_steps(n, 0), WR['c1'][1]))
                    if prv is not None:
                        release(prv)
                    xt_load(n, 2)
                    weave((scores_steps(n, 0), WR['c2'][0]),
                          (pv_steps(b, 3) + projqk_steps(n, 1)
                           + oproj_steps(b, range(0, 2))
                           + projvr_steps(n, 1), WR['c2'][1]))
                    xt_load(n, 3)
                    weave((scores_steps(n, 1), WR['c3'][0]),
                          (projqk_steps(n, 2) + oproj_steps(b, range(2, 6))
                           + projvr_steps(n, 2) + pv_steps(n, 0), WR['c3'][1]))
                    weave((scores_steps(n, 2), WR['c4'][0]),
                          (projqk_steps(n, 3) + oproj_steps(b, range(6, 10))
                           + projvr_steps(n, 3) + pv_steps(n, 1), WR['c4'][1]))
                    oproj(b, range(10, 12))
                else:
                    # last batch: spread its own o_proj through its attn;
                    # copies alternate ACT/DVE and the final pv block is split
                    # by query half so its o_proj tiles pipeline against the
                    # remaining pv matmuls.
                    pvk = ("pvps", b, 3)
                    weave((scores_steps(b, 3), WR['l1'][0]),
                          (oproj_steps(prv, range(12, 16), act_copy='mix')
                           + oproj_steps(b, range(0, 2))
                           + pv_steps(b, 2)
                           + oproj_steps(b, range(2, 5)), WR['l1'][1]))
                    if prv is not None:
                        release(prv)
                    weave((pv_steps(b, 3, (0, 256), keep_p=True, pvkey=pvk,
                                    pop_pv=False), 1),
                          (oproj_steps(b, range(5, 8), act_copy='mix'), 1))
                    weave((pv_steps(b, 3, (256, 512), pvkey=pvk), 1),
                          (oproj_steps(b, range(8, 12), act_copy='mix')
                           + oproj_steps(b, range(12, 14), act_copy='mix',
                                         alt=True), 2))
                    oproj(b, range(14, 16), act_copy='mix', alt=True)
                    release(b)

    nc.compile()
    return nc


_NC_CACHE = {}


def _get_nc(nb=B):
    if nb not in _NC_CACHE:
        _NC_CACHE[nb] = build_nc(nb)
    return _NC_CACHE[nb]


def _f8(a):
    return a.astype(F8NP)


def _host_prep(x, Wq, Wk, Wv, Wo):
    x2 = np.ascontiguousarray(x.reshape(BS, D))
    xs = (x2 * SX).astype(np.float32)
    xh = _f8(xs)
    xl = _f8(xs - xh.astype(np.float32))

    def xbt_layout(a):  # [BS, D] fp8 -> [128, 8, BS]
        return np.ascontiguousarray(a.reshape(BS, 8, 128).transpose(2, 1, 0))

    xbth = xbt_layout(xh)
    xbtl = xbt_layout(xl)

    half = 32
    inv_freq = 1.0 / (ROPE_THETA ** (np.arange(half, dtype=np.float64) / half))
    freqs = np.arange(S, dtype=np.float64)[:, None] * inv_freq[None, :]
    c_ = np.cos(freqs).astype(np.float32).T      # [32, S]
    s_ = np.sin(freqs).astype(np.float32).T
    cos1 = np.tile(c_, (4, 1))                        # [128, S]
    sins1 = np.vstack([-s_, s_, -s_, s_])             # [128, S]

    perm = np.zeros(128, dtype=np.int64)
    partner = np.zeros(128, dtype=np.int64)
    for hh in range(2):
        for j in range(64):
            perm[64 * hh + j] = 64 * hh + (2 * j if j < 32 else 2 * (j - 32) + 1)
            partner[64 * hh + j] = 64 * hh + (j + 32) % 64
    pswap = np.zeros((128, 128), dtype=np.float32)
    pswap[partner, np.arange(128)] = 1.0

    sinp1 = sins1[partner] / SCALE                    # u = ps * sinp trick
    cos1 = cos1 / SCALE                               # fold fp8 descale
    cos2 = np.ascontiguousarray(
        np.broadcast_to(cos1[:, None, :], (128, 2, S))).astype(BFNP)
    sinp2 = np.ascontiguousarray(
        np.broadcast_to(sinp1[:, None, :], (128, 2, S))).astype(BFNP)

    # maskw[p, j] = 1 if j >= p else 0, width 512 (cols >=128 all ones);
    # sliced to the exact exp'd range of each diagonal tile
    jj = np.arange(512)
    masksq = (jj[None, :] >= np.arange(128)[:, None]).astype(np.float32).astype(BFNP)

    def w_hilo(Wsl):  # [128 rows, D] (already permuted/sliced) -> hi,lo [128,8,128]
        ws = (Wsl.T * SW).astype(np.float32)          # [D, 128]
        wh = _f8(ws)
        wl = _f8(ws - wh.astype(np.float32))
        def lay(a):
            return np.ascontiguousarray(a.reshape(8, 128, 128).transpose(1, 0, 2))
        return lay(wh), lay(wl)

    in_maps = []
    for c in range(NCORES):
        sl = slice(128 * c, 128 * c + 128)
        wqh, wql = w_hilo(Wq[sl][perm])
        wkh, wkl = w_hilo(Wk[sl][perm])
        wvh, wvl = w_hilo(Wv[sl])

        def flat(a):
            return np.ascontiguousarray(a).reshape(128, -1)

        boot = np.concatenate(
            [flat(wqh), flat(wkh), flat(xbth[:, :, 0:512]), flat(xbtl[:, :, 0:512]),
             flat(wql), flat(wkl), flat(wvh), flat(wvl)], axis=1)
        in_maps.append({
            "xbth": xbth,
            "xbtl": xbtl,
            "boot": boot,
            "wot": np.ascontiguousarray(Wo[:, sl].T).astype(BFNP),
            "cos2": cos2,
            "sinp2": sinp2,
            "pswap": pswap.astype(BFNP),
            "masksq": masksq,
        })
    return in_maps


def kernel(x, Wq, Wk, Wv, Wo):
    global LAST_RESULTS
    x = np.asarray(x, dtype=np.float32)
    Wq = np.asarray(Wq, dtype=np.float32)
    Wk = np.asarray(Wk, dtype=np.float32)
    Wv = np.asarray(Wv, dtype=np.float32)
    Wo = np.asarray(Wo, dtype=np.float32)

    nc = _get_nc(B)
    in_maps = _host_prep(x, Wq, Wk, Wv, Wo)
    res = run_bass_kernel_spmd(nc, in_maps, core_ids=list(range(NCORES)),
                               trace=TRACE)
    LAST_RESULTS = res
    out = np.zeros((BS, D), dtype=np.float32)
    for c in range(NCORES):
        out += np.asarray(res.results[c]["y"]).astype(np.float32)
    return out.reshape(B, S, D)


# revision 46
# speedup vs baseline: 1.1041x; 1.0084x over previous
"""Causal MHA with RoPE on 8 Trainium2 NeuronCores.

Sharding: tensor-parallel over heads. Core c owns heads {2c, 2c+1} (a 128-wide
slice of the model dim). Each core computes Q/K/V projections for its heads,
full causal attention, and a partial o_proj; the host sums the 8 partial
outputs (the "all-reduce").

v3 (on top of the pipelined v2):
  - Q/K/V projections run in fp8e4 DoubleRow mode (4x PE throughput per
    column): host ships x and W as hi+lo fp8 pairs at a common power-of-2
    scale (sx*sw = 2^13); the 3-term product (xh*Wh + xl*Wh + xh*Wl) restores
    ~bf16 accuracy. The 2^13 descale is folded into the host RoPE tables for
    q/k and into the ones-rows value (denominator trick) for v.
  - softmax reciprocals read the PSUM ones-rows directly with
    partition-offset APs (no staging copies).
  - o_proj PSUM->SBUF copies run on the idle Pool engine; y DMAs issue from
    the SP queue.
  - prologue DMAs ordered so the first projection matmul starts ~3us in.

Device layouts (per core):
  x.T   [128 i-part, 8 i-tile, t]  fp8 hi+lo
  qk_sb [128 hd, 2(q/k), t] bf16;  hd = [head A (ev 0:32, od 32:64), head B]
  scores S.T [k, q] per 128-key tile; P = exp(S.T/8) bf16 in SBUF
  PV: v_sb k-tiles [VA(64) | SCALE-rows(64) | VB(64)]; scale-rows give denoms
  o_proj: ot [128 hd, t] bf16 (stationary) x Wo.T [128 hd, 1024] -> y bf16
"""
import sys
sys.path.insert(0, '/opt/trn_rl_repo')

import numpy as np
import ml_dtypes

import concourse.bass as bass
from concourse import bacc
import concourse.mybir as mybir
import concourse.tile as tile
from concourse.bass_utils import run_bass_kernel_spmd

BFNP = ml_dtypes.bfloat16
F8NP = ml_dtypes.float8_e4m3
F32 = mybir.dt.float32
BF16 = mybir.dt.bfloat16
FP8 = mybir.dt.float8e4
DR = mybir.MatmulPerfMode.DoubleRow
AF = mybir.ActivationFunctionType

B, S, D = 4, 2048, 1024
NCORES = 8
BS = B * S
ROPE_THETA = 10000.0

SX = 16.0          # x fp8 scale
SW = 512.0         # W fp8 scale
SCALE = SX * SW    # folded out via rope tables (q,k) and ones-rows (v)

TRACE = False
# queue per prologue DMA: sinp_a,cos_a,psw,msq,sinp_b,cos_b,
#                         xt01h,xt01l,xt02h,xt02l,xt03h,xt03l,wot
# a=ACT  s=SP(sync)  g=gpsimd(Pool)
PROLOGUE_QUEUES = "ggggggaaggssg"
BOOT_QUEUE = 's'
# weave ratios (score_per_round, other_per_round) per phase
WR = {'p1': (2, 2), 'p2': (2, 1), 'c1': (2, 1), 'c2': (1, 3), 'c3': (1, 3),
      'c4': (1, 1), 'l1': (3, 1)}
LAST_RESULTS = None
PE_LABELS = []
PE_LABEL_BY_NAME = {}


def build_nc(nb=B):
    global PE_LABELS
    PE_LABELS = []
    nc = bacc.Bacc()

    _mm = nc.tensor.matmul
    def _mm_tagged(*a, _label="?", **k):
        PE_LABELS.append(_label)
        r = _mm(*a, **k)
        PE_LABEL_BY_NAME[r.ins.name] = _label
        return r
    nc.tensor.matmul = _mm_tagged
    xbth = nc.dram_tensor("xbth", [128, 8, BS], FP8, kind="ExternalInput")
    xbtl = nc.dram_tensor("xbtl", [128, 8, BS], FP8, kind="ExternalInput")
    # boot: [wq_h|wk_h|xth00|xtl00|wq_l|wk_l|wv_h|wv_l] packed so the first
    # chunk's full working set arrives in 3 ordered DMAs on one queue
    boot = nc.dram_tensor("boot", [128, 14336], FP8, kind="ExternalInput")
    wot = nc.dram_tensor("wot", [128, D], BF16, kind="ExternalInput")
    cos2 = nc.dram_tensor("cos2", [128, 2, S], BF16, kind="ExternalInput")
    sinp2 = nc.dram_tensor("sinp2", [128, 2, S], BF16, kind="ExternalInput")
    pswap = nc.dram_tensor("pswap", [128, 128], BF16, kind="ExternalInput")
    masksq = nc.dram_tensor("masksq", [128, 512], BF16, kind="ExternalInput")
    y = nc.dram_tensor("y", [BS, D], BF16, kind="ExternalOutput")

    with tile.TileContext(nc) as tc:
        with tc.tile_pool(name="const", bufs=1) as constp, \
             tc.tile_pool(name="xt", bufs=4) as xtp, \
             tc.tile_pool(name="qk", bufs=2) as qkp, \
             tc.tile_pool(name="vsb", bufs=2) as vsp, \
             tc.tile_pool(name="u", bufs=4) as up, \
             tc.tile_pool(name="ptile", bufs=34) as pp, \
             tc.tile_pool(name="otp", bufs=2) as otp, \
             tc.tile_pool(name="rc", bufs=4) as rcp, \
             tc.tile_pool(name="yout", bufs=7) as yop, \
             tc.tile_pool(name="psum", bufs=1, space="PSUM") as psp:

            # ---- constant tiles (DMAs emitted in the prologue below) ----
            boot_sb = constp.tile([128, 14336], FP8)

            def bview(o, n, t):  # [128, n*t] slice -> [128, n, t]
                return boot_sb[:, o:o + n * t].rearrange("p (a c) -> p a c", a=n)

            wq_h = bview(0, 8, 128)
            wk_h = bview(1024, 8, 128)
            xth00 = bview(2048, 8, 512)
            xtl00 = bview(6144, 8, 512)
            wq_l = bview(10240, 8, 128)
            wk_l = bview(11264, 8, 128)
            wv_h = bview(12288, 8, 128)
            wv_l = bview(13312, 8, 128)
            sinp_a = constp.tile([128, 2, 512], BF16)
            cos_a = constp.tile([128, 2, 512], BF16)
            sinp_b = constp.tile([128, 2, S - 512], BF16)
            cos_b = constp.tile([128, 2, S - 512], BF16)
            psw_sb = constp.tile([128, 128], BF16)
            msq_sb = constp.tile([128, 512], BF16)
            wot_sb = constp.tile([128, D], BF16)
            warm = constp.tile([128, 2], F32)

            # ---- per-batch state (bufs=2 pools ring across batches) ----
            state = {}

            def xt_load(b, c, eng=None, enl=None):
                tb0 = (b % B) * S + 512 * c
                xth = xtp.tile([128, 8, 512], FP8, tag="xth", name=f"xth{b}_{c}")
                xtl = xtp.tile([128, 8, 512], FP8, tag="xtl", name=f"xtl{b}_{c}")
                (eng or nc.sync).dma_start(out=xth, in_=xbth[:, :, tb0:tb0 + 512])
                (enl or eng or nc.sync).dma_start(out=xtl, in_=xbtl[:, :, tb0:tb0 + 512])
                state[("xt", b, c)] = (xth, xtl)

            def projqk_a(b, c, a):
                """PE: 12 DoubleRow mms for q or k; DVE u/cc muls after a=1."""
                if ("qtr", b) not in state:
                    state[("qtr", b)] = qkp.tile([128, S], BF16, tag="qtr", name=f"qtr{b}")
                    state[("ktr", b)] = qkp.tile([128, S], BF16, tag="ktr", name=f"ktr{b}")
                    v = vsp.tile([128, 16, 192], BF16, tag="v", name=f"v{b}")
                    nc.gpsimd.memset(v[:, :, 64:128], SCALE)
                    state[("v", b)] = v
                t0 = 512 * c
                xth, xtl = state[("xt", b, c)]
                w_h, w_l = (wq_h, wq_l) if a == 0 else (wk_h, wk_l)
                ps = psp.tile([128, 512], F32, tag="proj", bufs=2, name=f"qk{b}_{c}_{a}")
                nmm = 0
                for xs, ws in ((xth, w_h), (xtl, w_h), (xth, w_l)):
                    for m in range(4):
                        nc.tensor.matmul(ps, ws[:, 2 * m:2 * m + 2, :],
                                         xs[:, 2 * m:2 * m + 2, :],
                                         start=(nmm == 0), stop=(nmm == 11),
                                         perf_mode=DR,
                                         _label=f"proj{'QK'[a]} b{b} c{c} m{nmm}")
                        nmm += 1
                state[("qkps", b, c, a)] = ps
                if a == 1:
                    u_sb = up.tile([128, 2, 512], BF16, tag="u")
                    cc_sb = up.tile([128, 2, 512], BF16, tag="cc")
                    sinp_t = sinp_a if c == 0 else sinp_b[:, :, t0 - 512:t0]
                    cos_t = cos_a if c == 0 else cos_b[:, :, t0 - 512:t0]
                    for aa in range(2):
                        qk_ps = state.pop(("qkps", b, c, aa))
                        nc.vector.tensor_mul(u_sb[:, aa, :], qk_ps, sinp_t[:, aa, :])
                        nc.vector.tensor_mul(cc_sb[:, aa, :], qk_ps, cos_t[:, aa, :])
                    state[("ucc", b, c)] = (u_sb, cc_sb)

            def projqk_steps(b, c):
                return [lambda: projqk_a(b, c, 0), lambda: projqk_a(b, c, 1)]

            def proj_qk(b, c):
                for s in projqk_steps(b, c):
                    s()

            def projvr_vt(b, c, tts):
                """PE: DoubleRow v mms for t-tiles tts."""
                xth, xtl = state[("xt", b, c)]
                if ("vtps", b, c) not in state:
                    state[("vtps", b, c)] = psp.tile([128, 512], F32, tag="proj",
                                                     bufs=2, name=f"vt{b}_{c}")
                vt_ps = state[("vtps", b, c)]
                for tt in tts:
                    nmm = 0
                    for xs, ws in ((xth, wv_h), (xtl, wv_h), (xth, wv_l)):
                        for m in range(4):
                            nc.tensor.matmul(vt_ps[:, 128 * tt:128 * tt + 128],
                                             xs[:, 2 * m:2 * m + 2, 128 * tt:128 * tt + 128],
                                             ws[:, 2 * m:2 * m + 2, :],
                                             start=(nmm == 0), stop=(nmm == 11),
                                             perf_mode=DR,
                                             _label=f"projV b{b} c{c} t{tt} m{nmm}")
                            nmm += 1

            def projvr_fin(b, c):
                """PE: 2 swap mms. DVE: rope adds + v copies."""
                v_sb = state[("v", b)]
                qk_dst = (state[("qtr", b)], state[("ktr", b)])
                t0 = 512 * c
                state.pop(("xt", b, c))
                u_sb, cc_sb = state.pop(("ucc", b, c))
                vt_ps = state.pop(("vtps", b, c))
                for a in range(2):
                    if a == 0:
                        sw_ps = psp.tile([128, 512], F32, tag="pv", bufs=1, name=f"sw{b}_{c}_{a}")
                    else:
                        sw_ps = psp.tile([128, 512], F32, tag="proj", bufs=2, name=f"sw{b}_{c}_{a}")
                    nc.tensor.matmul(sw_ps, psw_sb, u_sb[:, a, :],
                                     start=True, stop=True, _label=f"swap b{b} c{c} a{a}")
                    nc.vector.tensor_add(qk_dst[a][:, t0:t0 + 512], sw_ps, cc_sb[:, a, :])
                # v: [tok%128, tt, hd] -> v_sb ktiles [VA(64) | SCALE | VB(64)]
                vv = vt_ps.rearrange("p (t c) -> p t c", t=4)
                nc.vector.tensor_copy(v_sb[:, 4 * c:4 * c + 4, 0:64], vv[:, :, 0:64])
                nc.vector.tensor_copy(v_sb[:, 4 * c:4 * c + 4, 128:192], vv[:, :, 64:128])

            def projvr_steps(b, c):
                return [lambda: projvr_vt(b, c, (0, 1)),
                        lambda: projvr_vt(b, c, (2, 3)),
                        lambda: projvr_fin(b, c)]

            def proj_vr(b, c):
                for s in projvr_steps(b, c):
                    s()

            def proj_chunk(b, c):
                proj_qk(b, c)
                proj_vr(b, c)

            def weave(*pairs):
                """pairs of (steps, n_per_round): round-robin emission."""
                lists = [list(s) for s, _ in pairs]
                counts = [n for _, n in pairs]
                while any(lists):
                    for li, n in zip(lists, counts):
                        for _ in range(n):
                            if li:
                                li.pop(0)()

            def score_tile(b, qc, kp, hh):
                """One (kp, hh) tile: PE 2 mms -> ACT exp(s) -> Pool mask."""
                qtr, ktr = state[("qtr", b)], state[("ktr", b)]
                q0 = 512 * qc
                diag = kp >= 2 * qc
                h0 = 64 * hh
                p_t = pp.tile([128, 1024], BF16, tag="p", name=f"p{b}_{qc}_{kp}_{hh}")
                st = psp.tile([128, 1024], F32, tag="st", bufs=2, name=f"st{b}_{qc}_{kp}_{hh}")
                for j in range(2):
                    ki = 2 * kp + j
                    d = ki - 4 * qc
                    trim = 128 * d if diag else 0
                    nc.tensor.matmul(
                        st[:, 512 * j + trim:512 * j + 512],
                        ktr[h0:h0 + 64, 128 * ki:128 * ki + 128],
                        qtr[h0:h0 + 64, q0 + trim:q0 + 512],
                        start=True, stop=True,
                        _label=f"score b{b} q{qc} kp{kp} h{hh} j{j}")
                    if diag:
                        nc.scalar.activation(
                            p_t[:, 512 * j + trim:512 * j + 512],
                            st[:, 512 * j + trim:512 * j + 512],
                            AF.Exp, scale=0.125)
                        # mask only bites in the first 128 cols
                        # (jcol >= p is trivially true beyond)
                        nc.gpsimd.tensor_mul(
                            p_t[:, 512 * j + trim:512 * j + trim + 128],
                            p_t[:, 512 * j + trim:512 * j + trim + 128],
                            msq_sb[:, 0:128])
                if not diag:
                    nc.scalar.activation(p_t, st, AF.Exp, scale=0.125)
                state[("p", b, qc, kp, hh)] = p_t

            def scores_steps(b, qc):
                return [(lambda kp=kp, hh=hh: score_tile(b, qc, kp, hh))
                        for kp in range(2 * (qc + 1)) for hh in range(2)]

            def scores_block(b, qc):
                """PE: scores mms (trimmed on diagonal). ACT: exps. Pool: masks."""
                for s in scores_steps(b, qc):
                    s()

            def pv_hh(b, qc, hh, cols, pvkey):
                """PE: one head's pv accumulation group for query cols."""
                if ("ot", b) not in state:
                    state[("ot", b)] = otp.tile([128, S], BF16, tag="ot", name=f"ot{b}")
                v_sb = state[("v", b)]
                c0, c1 = cols
                nk = 4 * qc + 4
                if pvkey not in state:
                    state[pvkey] = psp.tile([128, 1024], F32, tag="pv", bufs=1,
                                            name=f"pv{b}_{qc}_{c0}")
                pv = state[pvkey]
                col0 = 0 if hh == 0 else 64
                kis = [ki for ki in range(nk)
                       if max(128 * (ki - 4 * qc), c0) < c1]
                for ii, ki in enumerate(kis):
                    d = ki - 4 * qc
                    trim = max(128 * d if d >= 0 else 0, c0)
                    kp, j = divmod(ki, 2)
                    p_t = state[("p", b, qc, kp, hh)]
                    nc.tensor.matmul(
                        pv[:, 512 * hh + trim:512 * hh + c1],
                        v_sb[:, ki, col0:col0 + 128],
                        p_t[:, 512 * j + trim:512 * j + c1],
                        start=(ii == 0), stop=(ii == len(kis) - 1),
                        _label=f"pv b{b} q{qc} h{hh} ki{ki}")

            def pv_norm(b, qc, cols, pvkey, keep_p, pop_pv=True):
                """DVE: 2 recips + 2 muls -> ot (denominators sit replicated
                in the ones-rows: hh=0 rows 64:128, hh=1 rows 0:64)."""
                ot = state[("ot", b)]
                c0, c1 = cols
                q0 = 512 * qc
                pv = state.pop(pvkey) if pop_pv else state[pvkey]
                if not keep_p:
                    for kp in range(2 * (qc + 1)):
                        for hh in range(2):
                            state.pop(("p", b, qc, kp, hh))
                r2 = rcp.tile([128, 512], F32, tag="r2")
                nc.vector.reciprocal(r2[0:64, c0:c1], pv[64:128, c0:c1])
                nc.vector.reciprocal(r2[64:128, c0:c1], pv[0:64, 512 + c0:512 + c1])
                nc.vector.tensor_mul(ot[0:64, q0 + c0:q0 + c1], pv[0:64, c0:c1],
                                     r2[0:64, c0:c1])
                nc.vector.tensor_mul(ot[64:128, q0 + c0:q0 + c1],
                                     pv[64:128, 512 + c0:512 + c1],
                                     r2[64:128, c0:c1])

            def pv_pair(b, qc, kp, pvkey):
                """pv mms for key-pair kp (both heads), chasing its exps; each
                head's group gets its own proj-tag psum tile (that ring is
                idle in the last cycle) so both stay open across the
                interleaved score tiles."""
                if ("ot", b) not in state:
                    state[("ot", b)] = otp.tile([128, S], BF16, tag="ot", name=f"ot{b}")
                v_sb = state[("v", b)]
                nk = 4 * qc + 4
                if (pvkey, 0) not in state:
                    for hh in range(2):
                        state[(pvkey, hh)] = psp.tile(
                            [128, 512], F32, tag="proj", bufs=2,
                            name=f"pvch{b}_{qc}_{hh}")
                for hh in range(2):
                    pv = state[(pvkey, hh)]
                    col0 = 0 if hh == 0 else 64
                    for j in range(2):
                        ki = 2 * kp + j
                        d = ki - 4 * qc
                        trim = 128 * d if d >= 0 else 0
                        p_t = state[("p", b, qc, kp, hh)]
                        nc.tensor.matmul(
                            pv[:, trim:512],
                            v_sb[:, ki, col0:col0 + 128],
                            p_t[:, 512 * j + trim:512 * j + 512],
                            start=(ki == 0), stop=(ki == nk - 1),
                            _label=f"pv b{b} q{qc} h{hh} ki{ki}")
                for hh in range(2):
                    state.pop(("p", b, qc, kp, hh))

            def chase_norm(b, qc, cols, pvkey, pop_pv=True):
                """DVE: recips + muls -> ot from the two chase psum tiles."""
                ot = state[("ot", b)]
                c0, c1 = cols
                q0 = 512 * qc
                pvA = state[(pvkey, 0)]
                pvB = state[(pvkey, 1)]
                if pop_pv:
                    state.pop((pvkey, 0))
                    state.pop((pvkey, 1))
                r2 = rcp.tile([128, 512], F32, tag="r2")
                nc.vector.reciprocal(r2[0:64, c0:c1], pvA[64:128, c0:c1])
                nc.vector.reciprocal(r2[64:128, c0:c1], pvB[0:64, c0:c1])
                nc.vector.tensor_mul(ot[0:64, q0 + c0:q0 + c1], pvA[0:64, c0:c1],
                                     r2[0:64, c0:c1])
                nc.vector.tensor_mul(ot[64:128, q0 + c0:q0 + c1],
                                     pvB[64:128, c0:c1], r2[64:128, c0:c1])

            def chase_steps(b, qc, pvkey):
                steps = []
                for kp in range(2 * (qc + 1)):
                    steps.append(lambda kp=kp: score_tile(b, qc, kp, 0))
                    steps.append(lambda kp=kp: score_tile(b, qc, kp, 1))
                    steps.append(lambda kp=kp: pv_pair(b, qc, kp, pvkey))
                return steps

            def pv_steps(b, qc, cols=(0, 512), keep_p=False, pvkey=None, pop_pv=True):
                if pvkey is None:
                    pvkey = ("pvps", b, qc)
                return [lambda: pv_hh(b, qc, 0, cols, pvkey),
                        lambda: pv_hh(b, qc, 1, cols, pvkey),
                        lambda: pv_norm(b, qc, cols, pvkey, keep_p, pop_pv)]

            def pv_block(b, qc, cols=(0, 512), keep_p=False, pvkey=None, pop_pv=True):
                for s in pv_steps(b, qc, cols, keep_p, pvkey, pop_pv):
                    s()

            def oproj_tile(b, tt, alt=False, act_copy=None):
                """PE: 2 mms into one wide tile; ACT-or-DVE wide copy;
                y DMA on SP queue."""
                ot = state[("ot", b)]
                tb0 = (b % B) * S
                yo = yop.tile([128, 1024], BF16, tag="yo")
                if alt and tt % 2 == 1:
                    op_ps = psp.tile([128, 1024], F32, tag="pv", bufs=1, name=f"op{b}_{tt}")
                else:
                    op_ps = psp.tile([128, 1024], F32, tag="st", bufs=2, name=f"op{b}_{tt}")
                for oc in range(2):
                    nc.tensor.matmul(op_ps[:, 512 * oc:512 * oc + 512],
                                     ot[:, 128 * tt:128 * tt + 128],
                                     wot_sb[:, 512 * oc:512 * oc + 512],
                                     start=True, stop=True,
                                     _label=f"oproj b{b} t{tt} o{oc}")
                if act_copy == 'mix':
                    on_act = (tt % 2 == 0)
                else:
                    on_act = (tt % 4 == 0) if act_copy is None else act_copy
                if on_act:
                    nc.scalar.activation(yo, op_ps, AF.Copy)
                else:
                    nc.vector.tensor_copy(yo, op_ps)
                nc.sync.dma_start(out=y[tb0 + 128 * tt:tb0 + 128 * tt + 128, :], in_=yo)

            def oproj_steps(b, tts, alt=False, act_copy=None):
                if b is None or b < 0:
                    return []
                return [(lambda tt=tt: oproj_tile(b, tt, alt, act_copy))
                        for tt in tts]

            def oproj(b, tts, alt=False, act_copy=None):
                for s in oproj_steps(b, tts, alt, act_copy):
                    s()

            def release(b):
                state.pop(("qtr", b))
                state.pop(("ktr", b))
                state.pop(("v", b))
                state.pop(("ot", b))

            # ---- pipelined emission ----
            # steady state per batch n (prev = n-1):
            #  [S2 V1] [P0n S3 V2] [V3 P1n] [O P2n] [P3n S0n] [S1n V0n]
            # prologue: the first-chunk working set (boot) wins the DMA pipe
            # in exact need-order on one queue; tables trail on gpsimd/sync.
            bq = {'a': nc.scalar, 's': nc.sync, 'g': nc.gpsimd}[BOOT_QUEUE]
            bq.dma_start(out=boot_sb[:, 0:6144], in_=boot[:, 0:6144])
            bq.dma_start(out=boot_sb[:, 6144:10240], in_=boot[:, 6144:10240])
            bq.dma_start(out=boot_sb[:, 10240:14336], in_=boot[:, 10240:14336])
            state[("xt", 0, 0)] = (xth00, xtl00)
            qmap = {'a': nc.scalar, 's': nc.sync, 'g': nc.gpsimd}
            pq = [qmap[ch] for ch in PROLOGUE_QUEUES]
            pq[0].dma_start(out=sinp_a, in_=sinp2[:, :, 0:512])
            pq[1].dma_start(out=cos_a, in_=cos2[:, :, 0:512])
            pq[2].dma_start(out=psw_sb, in_=pswap[:, :])
            pq[3].dma_start(out=msq_sb, in_=masksq[:, :])
            pq[4].dma_start(out=sinp_b, in_=sinp2[:, :, 512:S])
            pq[5].dma_start(out=cos_b, in_=cos2[:, :, 512:S])
            xt_load(0, 1, eng=pq[6], enl=pq[7])
            xt_load(0, 2, eng=pq[8], enl=pq[9])
            xt_load(0, 3, eng=pq[10], enl=pq[11])
            pq[12].dma_start(out=wot_sb, in_=wot[:, :])
            nc.scalar.activation(warm, psw_sb[:, 0:2], AF.Exp)

            # merged pipeline: per batch-cycle, interleave attn(b) blocks
            # with proj(b+1) sections and oproj(b-1) pairs so every engine
            # sees a mixed diet continuously. Attention on chunk 0 starts as
            # soon as its projection lands to cover the x-DMA-bound prologue.
            proj_chunk(0, 0)
            weave((scores_steps(0, 0), 1),
                  (projqk_steps(0, 1) + projvr_steps(0, 1), 1))
            weave((scores_steps(0, 1), WR['p1'][0]),
                  (pv_steps(0, 0) + projqk_steps(0, 2) + projvr_steps(0, 2), WR['p1'][1]))
            weave((scores_steps(0, 2), WR['p2'][0]),
                  (pv_steps(0, 1) + projqk_steps(0, 3) + projvr_steps(0, 3), WR['p2'][1]))

            # steady cycles: cycle b finishes attn(b), runs proj(b+1),
            # starts attn(b+1) through qc2/V1, and drains oproj(b-1)/oproj(b).
            # Score tiles (ACT-heavy) are woven between PE-heavy steps so the
            # exp stream pipelines behind matmuls instead of stalling the
            # 2-slot st psum ring.
            for b in range(nb):
                n = b + 1 if b + 1 < nb else None
                prv = b - 1 if b > 0 else None
                if n is not None:
                    xt_load(n, 0)
                    xt_load(n, 1)
                    weave((scores_steps(b, 3), WR['c1'][0]),
                          (pv_steps(b, 2) + projqk_steps(n, 0)
                           + oproj_steps(prv, range(12, 16))
                           + projvr_steps(n, 0), WR['c1'][1]))
                    if prv is not None:
                        release(prv)
                    xt_load(n, 2)
                    weave((scores_steps(n, 0), WR['c2'][0]),
                          (pv_steps(b, 3) + projqk_steps(n, 1)
                           + oproj_steps(b, range(0, 2))
                           + projvr_steps(n, 1), WR['c2'][1]))
                    xt_load(n, 3)
                    weave((scores_steps(n, 1), WR['c3'][0]),
                          (projqk_steps(n, 2) + oproj_steps(b, range(2, 6))
                           + projvr_steps(n, 2) + pv_steps(n, 0), WR['c3'][1]))
                    weave((scores_steps(n, 2), WR['c4'][0]),
                          (projqk_steps(n, 3) + oproj_steps(b, range(6, 10))
                           + projvr_steps(n, 3) + pv_steps(n, 1), WR['c4'][1]))
                    oproj(b, range(10, 12))
                else:
                    # last batch: each score tile's pv matmuls chase its exp
                    # (both pv head-groups stay open in separate psum banks),
                    # o_proj fills the gaps, and the qc3 normalization is
                    # split by query half so the final o_proj tiles pipeline.
                    pvk = ("pvps", b, 3)
                    weave((chase_steps(b, 3, pvk), WR['l1'][0]),
                          (oproj_steps(prv, range(12, 16), act_copy='mix')
                           + oproj_steps(b, range(0, 2))
                           + pv_steps(b, 2)
                           + oproj_steps(b, range(2, 8), act_copy='mix'), WR['l1'][1]))
                    if prv is not None:
                        release(prv)
                    chase_norm(b, 3, (0, 256), pvk, pop_pv=False)
                    oproj(b, range(8, 12), act_copy='mix')
                    chase_norm(b, 3, (256, 512), pvk)
                    oproj(b, range(12, 14), act_copy='mix', alt=True)
                    oproj(b, range(14, 16), act_copy='mix', alt=True)
                    release(b)

    nc.compile()
    return nc


_NC_CACHE = {}


def _get_nc(nb=B):
    if nb not in _NC_CACHE:
        _NC_CACHE[nb] = build_nc(nb)
    return _NC_CACHE[nb]


def _f8(a):
    return a.astype(F8NP)


def _host_prep(x, Wq, Wk, Wv, Wo):
    x2 = np.ascontiguousarray(x.reshape(BS, D))
    xs = (x2 * SX).astype(np.float32)
    xh = _f8(xs)
    xl = _f8(xs - xh.astype(np.float32))

    def xbt_layout(a):  # [BS, D] fp8 -> [128, 8, BS]
        return np.ascontiguousarray(a.reshape(BS, 8, 128).transpose(2, 1, 0))

    xbth = xbt_layout(xh)
    xbtl = xbt_layout(xl)

    half = 32
    inv_freq = 1.0 / (ROPE_THETA ** (np.arange(half, dtype=np.float64) / half))
    freqs = np.arange(S, dtype=np.float64)[:, None] * inv_freq[None, :]
    c_ = np.cos(freqs).astype(np.float32).T      # [32, S]
    s_ = np.sin(freqs).astype(np.float32).T
    cos1 = np.tile(c_, (4, 1))                        # [128, S]
    sins1 = np.vstack([-s_, s_, -s_, s_])             # [128, S]

    perm = np.zeros(128, dtype=np.int64)
    partner = np.zeros(128, dtype=np.int64)
    for hh in range(2):
        for j in range(64):
            perm[64 * hh + j] = 64 * hh + (2 * j if j < 32 else 2 * (j - 32) + 1)
            partner[64 * hh + j] = 64 * hh + (j + 32) % 64
    pswap = np.zeros((128, 128), dtype=np.float32)
    pswap[partner, np.arange(128)] = 1.0

    sinp1 = sins1[partner] / SCALE                    # u = ps * sinp trick
    cos1 = cos1 / SCALE                               # fold fp8 descale
    cos2 = np.ascontiguousarray(
        np.broadcast_to(cos1[:, None, :], (128, 2, S))).astype(BFNP)
    sinp2 = np.ascontiguousarray(
        np.broadcast_to(sinp1[:, None, :], (128, 2, S))).astype(BFNP)

    # maskw[p, j] = 1 if j >= p else 0, width 512 (cols >=128 all ones);
    # sliced to the exact exp'd range of each diagonal tile
    jj = np.arange(512)
    masksq = (jj[None, :] >= np.arange(128)[:, None]).astype(np.float32).astype(BFNP)

    def w_hilo(Wsl):  # [128 rows, D] (already permuted/sliced) -> hi,lo [128,8,128]
        ws = (Wsl.T * SW).astype(np.float32)          # [D, 128]
        wh = _f8(ws)
        wl = _f8(ws - wh.astype(np.float32))
        def lay(a):
            return np.ascontiguousarray(a.reshape(8, 128, 128).transpose(1, 0, 2))
        return lay(wh), lay(wl)

    in_maps = []
    for c in range(NCORES):
        sl = slice(128 * c, 128 * c + 128)
        wqh, wql = w_hilo(Wq[sl][perm])
        wkh, wkl = w_hilo(Wk[sl][perm])
        wvh, wvl = w_hilo(Wv[sl])

        def flat(a):
            return np.ascontiguousarray(a).reshape(128, -1)

        boot = np.concatenate(
            [flat(wqh), flat(wkh), flat(xbth[:, :, 0:512]), flat(xbtl[:, :, 0:512]),
             flat(wql), flat(wkl), flat(wvh), flat(wvl)], axis=1)
        in_maps.append({
            "xbth": xbth,
            "xbtl": xbtl,
            "boot": boot,
            "wot": np.ascontiguousarray(Wo[:, sl].T).astype(BFNP),
            "cos2": cos2,
            "sinp2": sinp2,
            "pswap": pswap.astype(BFNP),
            "masksq": masksq,
        })
    return in_maps


def kernel(x, Wq, Wk, Wv, Wo):
    global LAST_RESULTS
    x = np.asarray(x, dtype=np.float32)
    Wq = np.asarray(Wq, dtype=np.float32)
    Wk = np.asarray(Wk, dtype=np.float32)
    Wv = np.asarray(Wv, dtype=np.float32)
    Wo = np.asarray(Wo, dtype=np.float32)

    nc = _get_nc(B)
    in_maps = _host_prep(x, Wq, Wk, Wv, Wo)
    res = run_bass_kernel_spmd(nc, in_maps, core_ids=list(range(NCORES)),
                               trace=TRACE)
    LAST_RESULTS = res
    out = np.zeros((BS, D), dtype=np.float32)
    for c in range(NCORES):
        out += np.asarray(res.results[c]["y"]).astype(np.float32)
    return out.reshape(B, S, D)


# revision 48
# speedup vs baseline: 1.1057x; 1.0015x over previous
"""Causal MHA with RoPE on 8 Trainium2 NeuronCores.

Sharding: tensor-parallel over heads. Core c owns heads {2c, 2c+1} (a 128-wide
slice of the model dim). Each core computes Q/K/V projections for its heads,
full causal attention, and a partial o_proj; the host sums the 8 partial
outputs (the "all-reduce").

v3 (on top of the pipelined v2):
  - Q/K/V projections run in fp8e4 DoubleRow mode (4x PE throughput per
    column): host ships x and W as hi+lo fp8 pairs at a common power-of-2
    scale (sx*sw = 2^13); the 3-term product (xh*Wh + xl*Wh + xh*Wl) restores
    ~bf16 accuracy. The 2^13 descale is folded into the host RoPE tables for
    q/k and into the ones-rows value (denominator trick) for v.
  - softmax reciprocals read the PSUM ones-rows directly with
    partition-offset APs (no staging copies).
  - o_proj PSUM->SBUF copies run on the idle Pool engine; y DMAs issue from
    the SP queue.
  - prologue DMAs ordered so the first projection matmul starts ~3us in.

Device layouts (per core):
  x.T   [128 i-part, 8 i-tile, t]  fp8 hi+lo
  qk_sb [128 hd, 2(q/k), t] bf16;  hd = [head A (ev 0:32, od 32:64), head B]
  scores S.T [k, q] per 128-key tile; P = exp(S.T/8) bf16 in SBUF
  PV: v_sb k-tiles [VA(64) | SCALE-rows(64) | VB(64)]; scale-rows give denoms
  o_proj: ot [128 hd, t] bf16 (stationary) x Wo.T [128 hd, 1024] -> y bf16
"""
import sys
sys.path.insert(0, '/opt/trn_rl_repo')

import numpy as np
import ml_dtypes

import concourse.bass as bass
from concourse import bacc
import concourse.mybir as mybir
import concourse.tile as tile
from concourse.bass_utils import run_bass_kernel_spmd

BFNP = ml_dtypes.bfloat16
F8NP = ml_dtypes.float8_e4m3
F32 = mybir.dt.float32
BF16 = mybir.dt.bfloat16
FP8 = mybir.dt.float8e4
DR = mybir.MatmulPerfMode.DoubleRow
AF = mybir.ActivationFunctionType

B, S, D = 4, 2048, 1024
NCORES = 8
BS = B * S
ROPE_THETA = 10000.0

SX = 16.0          # x fp8 scale
SW = 512.0         # W fp8 scale
SCALE = SX * SW    # folded out via rope tables (q,k) and ones-rows (v)

TRACE = False
# queue per prologue DMA: sinp_a,cos_a,psw,msq,sinp_b,cos_b,
#                         xt01h,xt01l,xt02h,xt02l,xt03h,xt03l,wot
# a=ACT  s=SP(sync)  g=gpsimd(Pool)
PROLOGUE_QUEUES = "ggggggaaggssg"
BOOT_QUEUE = 's'
# weave ratios (score_per_round, other_per_round) per phase
WR = {'p1': (2, 2), 'p2': (2, 1), 'c1': (2, 1), 'c2': (1, 3), 'c3': (1, 3),
      'c4': (1, 1), 'l1': (3, 1)}
# copy-engine per oproj site: None=tt%4 ACT, 'mix'=tt%2, True=ACT, False=DVE
OC = {'c1': 'mix', 'c2': None, 'c3': None, 'c4': False, 'c5': None,
      'l1': 'mix', 'l2': 'mix', 'l3': 'mix'}
LAST_RESULTS = None
PE_LABELS = []
PE_LABEL_BY_NAME = {}


def build_nc(nb=B):
    global PE_LABELS
    PE_LABELS = []
    nc = bacc.Bacc()

    _mm = nc.tensor.matmul
    def _mm_tagged(*a, _label="?", **k):
        PE_LABELS.append(_label)
        r = _mm(*a, **k)
        PE_LABEL_BY_NAME[r.ins.name] = _label
        return r
    nc.tensor.matmul = _mm_tagged
    xbth = nc.dram_tensor("xbth", [128, 8, BS], FP8, kind="ExternalInput")
    xbtl = nc.dram_tensor("xbtl", [128, 8, BS], FP8, kind="ExternalInput")
    # boot: [wq_h|wk_h|xth00|xtl00|wq_l|wk_l|wv_h|wv_l] packed so the first
    # chunk's full working set arrives in 3 ordered DMAs on one queue
    boot = nc.dram_tensor("boot", [128, 14336], FP8, kind="ExternalInput")
    wot = nc.dram_tensor("wot", [128, D], BF16, kind="ExternalInput")
    cos2 = nc.dram_tensor("cos2", [128, 2, S], BF16, kind="ExternalInput")
    sinp2 = nc.dram_tensor("sinp2", [128, 2, S], BF16, kind="ExternalInput")
    pswap = nc.dram_tensor("pswap", [128, 128], BF16, kind="ExternalInput")
    masksq = nc.dram_tensor("masksq", [128, 512], BF16, kind="ExternalInput")
    y = nc.dram_tensor("y", [BS, D], BF16, kind="ExternalOutput")

    with tile.TileContext(nc) as tc:
        with tc.tile_pool(name="const", bufs=1) as constp, \
             tc.tile_pool(name="xt", bufs=4) as xtp, \
             tc.tile_pool(name="qk", bufs=2) as qkp, \
             tc.tile_pool(name="vsb", bufs=2) as vsp, \
             tc.tile_pool(name="u", bufs=4) as up, \
             tc.tile_pool(name="ptile", bufs=34) as pp, \
             tc.tile_pool(name="otp", bufs=2) as otp, \
             tc.tile_pool(name="rc", bufs=4) as rcp, \
             tc.tile_pool(name="yout", bufs=7) as yop, \
             tc.tile_pool(name="psum", bufs=1, space="PSUM") as psp:

            # ---- constant tiles (DMAs emitted in the prologue below) ----
            boot_sb = constp.tile([128, 14336], FP8)

            def bview(o, n, t):  # [128, n*t] slice -> [128, n, t]
                return boot_sb[:, o:o + n * t].rearrange("p (a c) -> p a c", a=n)

            wq_h = bview(0, 8, 128)
            wk_h = bview(1024, 8, 128)
            xth00 = bview(2048, 8, 512)
            xtl00 = bview(6144, 8, 512)
            wq_l = bview(10240, 8, 128)
            wk_l = bview(11264, 8, 128)
            wv_h = bview(12288, 8, 128)
            wv_l = bview(13312, 8, 128)
            sinp_a = constp.tile([128, 2, 512], BF16)
            cos_a = constp.tile([128, 2, 512], BF16)
            sinp_b = constp.tile([128, 2, S - 512], BF16)
            cos_b = constp.tile([128, 2, S - 512], BF16)
            psw_sb = constp.tile([128, 128], BF16)
            msq_sb = constp.tile([128, 512], BF16)
            wot_sb = constp.tile([128, D], BF16)
            warm = constp.tile([128, 2], F32)

            # ---- per-batch state (bufs=2 pools ring across batches) ----
            state = {}

            def xt_load(b, c, eng=None, enl=None):
                tb0 = (b % B) * S + 512 * c
                xth = xtp.tile([128, 8, 512], FP8, tag="xth", name=f"xth{b}_{c}")
                xtl = xtp.tile([128, 8, 512], FP8, tag="xtl", name=f"xtl{b}_{c}")
                (eng or nc.sync).dma_start(out=xth, in_=xbth[:, :, tb0:tb0 + 512])
                (enl or eng or nc.sync).dma_start(out=xtl, in_=xbtl[:, :, tb0:tb0 + 512])
                state[("xt", b, c)] = (xth, xtl)

            def projqk_a(b, c, a):
                """PE: 12 DoubleRow mms for q or k; DVE u/cc muls after a=1."""
                if ("qtr", b) not in state:
                    state[("qtr", b)] = qkp.tile([128, S], BF16, tag="qtr", name=f"qtr{b}")
                    state[("ktr", b)] = qkp.tile([128, S], BF16, tag="ktr", name=f"ktr{b}")
                    v = vsp.tile([128, 16, 192], BF16, tag="v", name=f"v{b}")
                    nc.gpsimd.memset(v[:, :, 64:128], SCALE)
                    state[("v", b)] = v
                t0 = 512 * c
                xth, xtl = state[("xt", b, c)]
                w_h, w_l = (wq_h, wq_l) if a == 0 else (wk_h, wk_l)
                ps = psp.tile([128, 512], F32, tag="proj", bufs=2, name=f"qk{b}_{c}_{a}")
                nmm = 0
                for xs, ws in ((xth, w_h), (xtl, w_h), (xth, w_l)):
                    for m in range(4):
                        nc.tensor.matmul(ps, ws[:, 2 * m:2 * m + 2, :],
                                         xs[:, 2 * m:2 * m + 2, :],
                                         start=(nmm == 0), stop=(nmm == 11),
                                         perf_mode=DR,
                                         _label=f"proj{'QK'[a]} b{b} c{c} m{nmm}")
                        nmm += 1
                state[("qkps", b, c, a)] = ps
                if a == 1:
                    u_sb = up.tile([128, 2, 512], BF16, tag="u")
                    cc_sb = up.tile([128, 2, 512], BF16, tag="cc")
                    sinp_t = sinp_a if c == 0 else sinp_b[:, :, t0 - 512:t0]
                    cos_t = cos_a if c == 0 else cos_b[:, :, t0 - 512:t0]
                    for aa in range(2):
                        qk_ps = state.pop(("qkps", b, c, aa))
                        nc.vector.tensor_mul(u_sb[:, aa, :], qk_ps, sinp_t[:, aa, :])
                        nc.vector.tensor_mul(cc_sb[:, aa, :], qk_ps, cos_t[:, aa, :])
                    state[("ucc", b, c)] = (u_sb, cc_sb)

            def projqk_steps(b, c):
                return [lambda: projqk_a(b, c, 0), lambda: projqk_a(b, c, 1)]

            def proj_qk(b, c):
                for s in projqk_steps(b, c):
                    s()

            def projvr_vt(b, c, tts):
                """PE: DoubleRow v mms for t-tiles tts."""
                xth, xtl = state[("xt", b, c)]
                if ("vtps", b, c) not in state:
                    state[("vtps", b, c)] = psp.tile([128, 512], F32, tag="proj",
                                                     bufs=2, name=f"vt{b}_{c}")
                vt_ps = state[("vtps", b, c)]
                for tt in tts:
                    nmm = 0
                    for xs, ws in ((xth, wv_h), (xtl, wv_h), (xth, wv_l)):
                        for m in range(4):
                            nc.tensor.matmul(vt_ps[:, 128 * tt:128 * tt + 128],
                                             xs[:, 2 * m:2 * m + 2, 128 * tt:128 * tt + 128],
                                             ws[:, 2 * m:2 * m + 2, :],
                                             start=(nmm == 0), stop=(nmm == 11),
                                             perf_mode=DR,
                                             _label=f"projV b{b} c{c} t{tt} m{nmm}")
                            nmm += 1

            def projvr_fin(b, c):
                """PE: 2 swap mms. DVE: rope adds + v copies."""
                v_sb = state[("v", b)]
                qk_dst = (state[("qtr", b)], state[("ktr", b)])
                t0 = 512 * c
                state.pop(("xt", b, c))
                u_sb, cc_sb = state.pop(("ucc", b, c))
                vt_ps = state.pop(("vtps", b, c))
                for a in range(2):
                    if a == 0:
                        sw_ps = psp.tile([128, 512], F32, tag="pv", bufs=1, name=f"sw{b}_{c}_{a}")
                    else:
                        sw_ps = psp.tile([128, 512], F32, tag="proj", bufs=2, name=f"sw{b}_{c}_{a}")
                    nc.tensor.matmul(sw_ps, psw_sb, u_sb[:, a, :],
                                     start=True, stop=True, _label=f"swap b{b} c{c} a{a}")
                    nc.vector.tensor_add(qk_dst[a][:, t0:t0 + 512], sw_ps, cc_sb[:, a, :])
                # v: [tok%128, tt, hd] -> v_sb ktiles [VA(64) | SCALE | VB(64)]
                vv = vt_ps.rearrange("p (t c) -> p t c", t=4)
                nc.vector.tensor_copy(v_sb[:, 4 * c:4 * c + 4, 0:64], vv[:, :, 0:64])
                nc.vector.tensor_copy(v_sb[:, 4 * c:4 * c + 4, 128:192], vv[:, :, 64:128])

            def projvr_steps(b, c):
                return [lambda: projvr_vt(b, c, (0, 1)),
                        lambda: projvr_vt(b, c, (2, 3)),
                        lambda: projvr_fin(b, c)]

            def proj_vr(b, c):
                for s in projvr_steps(b, c):
                    s()

            def proj_chunk(b, c):
                proj_qk(b, c)
                proj_vr(b, c)

            def weave(*pairs):
                """pairs of (steps, n_per_round): round-robin emission."""
                lists = [list(s) for s, _ in pairs]
                counts = [n for _, n in pairs]
                while any(lists):
                    for li, n in zip(lists, counts):
                        for _ in range(n):
                            if li:
                                li.pop(0)()

            def score_tile(b, qc, kp, hh):
                """One (kp, hh) tile: PE 2 mms -> ACT exp(s) -> Pool mask."""
                qtr, ktr = state[("qtr", b)], state[("ktr", b)]
                q0 = 512 * qc
                diag = kp >= 2 * qc
                h0 = 64 * hh
                p_t = pp.tile([128, 1024], BF16, tag="p", name=f"p{b}_{qc}_{kp}_{hh}")
                st = psp.tile([128, 1024], F32, tag="st", bufs=2, name=f"st{b}_{qc}_{kp}_{hh}")
                for j in range(2):
                    ki = 2 * kp + j
                    d = ki - 4 * qc
                    trim = 128 * d if diag else 0
                    nc.tensor.matmul(
                        st[:, 512 * j + trim:512 * j + 512],
                        ktr[h0:h0 + 64, 128 * ki:128 * ki + 128],
                        qtr[h0:h0 + 64, q0 + trim:q0 + 512],
                        start=True, stop=True,
                        _label=f"score b{b} q{qc} kp{kp} h{hh} j{j}")
                    if diag:
                        nc.scalar.activation(
                            p_t[:, 512 * j + trim:512 * j + 512],
                            st[:, 512 * j + trim:512 * j + 512],
                            AF.Exp, scale=0.125)
                        # mask only bites in the first 128 cols
                        # (jcol >= p is trivially true beyond)
                        nc.gpsimd.tensor_mul(
                            p_t[:, 512 * j + trim:512 * j + trim + 128],
                            p_t[:, 512 * j + trim:512 * j + trim + 128],
                            msq_sb[:, 0:128])
                if not diag:
                    nc.scalar.activation(p_t, st, AF.Exp, scale=0.125)
                state[("p", b, qc, kp, hh)] = p_t

            def scores_steps(b, qc):
                return [(lambda kp=kp, hh=hh: score_tile(b, qc, kp, hh))
                        for kp in range(2 * (qc + 1)) for hh in range(2)]

            def scores_block(b, qc):
                """PE: scores mms (trimmed on diagonal). ACT: exps. Pool: masks."""
                for s in scores_steps(b, qc):
                    s()

            def pv_hh(b, qc, hh, cols, pvkey):
                """PE: one head's pv accumulation group for query cols."""
                if ("ot", b) not in state:
                    state[("ot", b)] = otp.tile([128, S], BF16, tag="ot", name=f"ot{b}")
                v_sb = state[("v", b)]
                c0, c1 = cols
                nk = 4 * qc + 4
                if pvkey not in state:
                    state[pvkey] = psp.tile([128, 1024], F32, tag="pv", bufs=1,
                                            name=f"pv{b}_{qc}_{c0}")
                pv = state[pvkey]
                col0 = 0 if hh == 0 else 64
                kis = [ki for ki in range(nk)
                       if max(128 * (ki - 4 * qc), c0) < c1]
                for ii, ki in enumerate(kis):
                    d = ki - 4 * qc
                    trim = max(128 * d if d >= 0 else 0, c0)
                    kp, j = divmod(ki, 2)
                    p_t = state[("p", b, qc, kp, hh)]
                    nc.tensor.matmul(
                        pv[:, 512 * hh + trim:512 * hh + c1],
                        v_sb[:, ki, col0:col0 + 128],
                        p_t[:, 512 * j + trim:512 * j + c1],
                        start=(ii == 0), stop=(ii == len(kis) - 1),
                        _label=f"pv b{b} q{qc} h{hh} ki{ki}")

            def pv_norm(b, qc, cols, pvkey, keep_p, pop_pv=True):
                """DVE: 2 recips + 2 muls -> ot (denominators sit replicated
                in the ones-rows: hh=0 rows 64:128, hh=1 rows 0:64)."""
                ot = state[("ot", b)]
                c0, c1 = cols
                q0 = 512 * qc
                pv = state.pop(pvkey) if pop_pv else state[pvkey]
                if not keep_p:
                    for kp in range(2 * (qc + 1)):
                        for hh in range(2):
                            state.pop(("p", b, qc, kp, hh))
                r2 = rcp.tile([128, 512], F32, tag="r2")
                nc.vector.reciprocal(r2[0:64, c0:c1], pv[64:128, c0:c1])
                nc.vector.reciprocal(r2[64:128, c0:c1], pv[0:64, 512 + c0:512 + c1])
                nc.vector.tensor_mul(ot[0:64, q0 + c0:q0 + c1], pv[0:64, c0:c1],
                                     r2[0:64, c0:c1])
                nc.vector.tensor_mul(ot[64:128, q0 + c0:q0 + c1],
                                     pv[64:128, 512 + c0:512 + c1],
                                     r2[64:128, c0:c1])

            def pv_pair(b, qc, kp, pvkey):
                """pv mms for key-pair kp (both heads), chasing its exps; each
                head's group gets its own proj-tag psum tile (that ring is
                idle in the last cycle) so both stay open across the
                interleaved score tiles."""
                if ("ot", b) not in state:
                    state[("ot", b)] = otp.tile([128, S], BF16, tag="ot", name=f"ot{b}")
                v_sb = state[("v", b)]
                nk = 4 * qc + 4
                if (pvkey, 0) not in state:
                    for hh in range(2):
                        state[(pvkey, hh)] = psp.tile(
                            [128, 512], F32, tag="proj", bufs=2,
                            name=f"pvch{b}_{qc}_{hh}")
                for hh in range(2):
                    pv = state[(pvkey, hh)]
                    col0 = 0 if hh == 0 else 64
                    for j in range(2):
                        ki = 2 * kp + j
                        d = ki - 4 * qc
                        trim = 128 * d if d >= 0 else 0
                        p_t = state[("p", b, qc, kp, hh)]
                        nc.tensor.matmul(
                            pv[:, trim:512],
                            v_sb[:, ki, col0:col0 + 128],
                            p_t[:, 512 * j + trim:512 * j + 512],
                            start=(ki == 0), stop=(ki == nk - 1),
                            _label=f"pv b{b} q{qc} h{hh} ki{ki}")
                for hh in range(2):
                    state.pop(("p", b, qc, kp, hh))

            def chase_norm(b, qc, cols, pvkey, pop_pv=True):
                """DVE: recips + muls -> ot from the two chase psum tiles."""
                ot = state[("ot", b)]
                c0, c1 = cols
                q0 = 512 * qc
                pvA = state[(pvkey, 0)]
                pvB = state[(pvkey, 1)]
                if pop_pv:
                    state.pop((pvkey, 0))
                    state.pop((pvkey, 1))
                r2 = rcp.tile([128, 512], F32, tag="r2")
                nc.vector.reciprocal(r2[0:64, c0:c1], pvA[64:128, c0:c1])
                nc.vector.reciprocal(r2[64:128, c0:c1], pvB[0:64, c0:c1])
                nc.vector.tensor_mul(ot[0:64, q0 + c0:q0 + c1], pvA[0:64, c0:c1],
                                     r2[0:64, c0:c1])
                nc.vector.tensor_mul(ot[64:128, q0 + c0:q0 + c1],
                                     pvB[64:128, c0:c1], r2[64:128, c0:c1])

            def chase_steps(b, qc, pvkey):
                steps = []
                for kp in range(2 * (qc + 1)):
                    steps.append(lambda kp=kp: score_tile(b, qc, kp, 0))
                    steps.append(lambda kp=kp: score_tile(b, qc, kp, 1))
                    steps.append(lambda kp=kp: pv_pair(b, qc, kp, pvkey))
                return steps

            def pv_steps(b, qc, cols=(0, 512), keep_p=False, pvkey=None, pop_pv=True):
                if pvkey is None:
                    pvkey = ("pvps", b, qc)
                return [lambda: pv_hh(b, qc, 0, cols, pvkey),
                        lambda: pv_hh(b, qc, 1, cols, pvkey),
                        lambda: pv_norm(b, qc, cols, pvkey, keep_p, pop_pv)]

            def pv_block(b, qc, cols=(0, 512), keep_p=False, pvkey=None, pop_pv=True):
                for s in pv_steps(b, qc, cols, keep_p, pvkey, pop_pv):
                    s()

            def oproj_tile(b, tt, alt=False, act_copy=None):
                """PE: 2 mms into one wide tile; ACT-or-DVE wide copy;
                y DMA on SP queue."""
                ot = state[("ot", b)]
                tb0 = (b % B) * S
                yo = yop.tile([128, 1024], BF16, tag="yo")
                if alt and tt % 2 == 1:
                    op_ps = psp.tile([128, 1024], F32, tag="pv", bufs=1, name=f"op{b}_{tt}")
                else:
                    op_ps = psp.tile([128, 1024], F32, tag="st", bufs=2, name=f"op{b}_{tt}")
                for oc in range(2):
                    nc.tensor.matmul(op_ps[:, 512 * oc:512 * oc + 512],
                                     ot[:, 128 * tt:128 * tt + 128],
                                     wot_sb[:, 512 * oc:512 * oc + 512],
                                     start=True, stop=True,
                                     _label=f"oproj b{b} t{tt} o{oc}")
                if act_copy == 'mix':
                    on_act = (tt % 2 == 0)
                else:
                    on_act = (tt % 4 == 0) if act_copy is None else act_copy
                if on_act:
                    nc.scalar.activation(yo, op_ps, AF.Copy)
                else:
                    nc.vector.tensor_copy(yo, op_ps)
                nc.sync.dma_start(out=y[tb0 + 128 * tt:tb0 + 128 * tt + 128, :], in_=yo)

            def oproj_steps(b, tts, alt=False, act_copy=None):
                if b is None or b < 0:
                    return []
                return [(lambda tt=tt: oproj_tile(b, tt, alt, act_copy))
                        for tt in tts]

            def oproj(b, tts, alt=False, act_copy=None):
                for s in oproj_steps(b, tts, alt, act_copy):
                    s()

            def release(b):
                state.pop(("qtr", b))
                state.pop(("ktr", b))
                state.pop(("v", b))
                state.pop(("ot", b))

            # ---- pipelined emission ----
            # steady state per batch n (prev = n-1):
            #  [S2 V1] [P0n S3 V2] [V3 P1n] [O P2n] [P3n S0n] [S1n V0n]
            # prologue: the first-chunk working set (boot) wins the DMA pipe
            # in exact need-order on one queue; tables trail on gpsimd/sync.
            bq = {'a': nc.scalar, 's': nc.sync, 'g': nc.gpsimd}[BOOT_QUEUE]
            bq.dma_start(out=boot_sb[:, 0:6144], in_=boot[:, 0:6144])
            bq.dma_start(out=boot_sb[:, 6144:10240], in_=boot[:, 6144:10240])
            bq.dma_start(out=boot_sb[:, 10240:14336], in_=boot[:, 10240:14336])
            state[("xt", 0, 0)] = (xth00, xtl00)
            qmap = {'a': nc.scalar, 's': nc.sync, 'g': nc.gpsimd}
            pq = [qmap[ch] for ch in PROLOGUE_QUEUES]
            pq[0].dma_start(out=sinp_a, in_=sinp2[:, :, 0:512])
            pq[1].dma_start(out=cos_a, in_=cos2[:, :, 0:512])
            pq[2].dma_start(out=psw_sb, in_=pswap[:, :])
            pq[3].dma_start(out=msq_sb, in_=masksq[:, :])
            pq[4].dma_start(out=sinp_b, in_=sinp2[:, :, 512:S])
            pq[5].dma_start(out=cos_b, in_=cos2[:, :, 512:S])
            xt_load(0, 1, eng=pq[6], enl=pq[7])
            xt_load(0, 2, eng=pq[8], enl=pq[9])
            xt_load(0, 3, eng=pq[10], enl=pq[11])
            pq[12].dma_start(out=wot_sb, in_=wot[:, :])
            nc.scalar.activation(warm, psw_sb[:, 0:2], AF.Exp)

            # merged pipeline: per batch-cycle, interleave attn(b) blocks
            # with proj(b+1) sections and oproj(b-1) pairs so every engine
            # sees a mixed diet continuously. Attention on chunk 0 starts as
            # soon as its projection lands to cover the x-DMA-bound prologue.
            proj_chunk(0, 0)
            weave((scores_steps(0, 0), 1),
                  (projqk_steps(0, 1) + projvr_steps(0, 1), 1))
            weave((scores_steps(0, 1), WR['p1'][0]),
                  (pv_steps(0, 0) + projqk_steps(0, 2) + projvr_steps(0, 2), WR['p1'][1]))
            weave((scores_steps(0, 2), WR['p2'][0]),
                  (pv_steps(0, 1) + projqk_steps(0, 3) + projvr_steps(0, 3), WR['p2'][1]))

            # steady cycles: cycle b finishes attn(b), runs proj(b+1),
            # starts attn(b+1) through qc2/V1, and drains oproj(b-1)/oproj(b).
            # Score tiles (ACT-heavy) are woven between PE-heavy steps so the
            # exp stream pipelines behind matmuls instead of stalling the
            # 2-slot st psum ring.
            for b in range(nb):
                n = b + 1 if b + 1 < nb else None
                prv = b - 1 if b > 0 else None
                if n is not None:
                    xt_load(n, 0)
                    xt_load(n, 1)
                    weave((scores_steps(b, 3), WR['c1'][0]),
                          (pv_steps(b, 2) + projqk_steps(n, 0)
                           + oproj_steps(prv, range(12, 16), act_copy=OC['c1'])
                           + projvr_steps(n, 0), WR['c1'][1]))
                    if prv is not None:
                        release(prv)
                    xt_load(n, 2)
                    weave((scores_steps(n, 0), WR['c2'][0]),
                          (pv_steps(b, 3) + projqk_steps(n, 1)
                           + oproj_steps(b, range(0, 2), act_copy=OC['c2'])
                           + projvr_steps(n, 1), WR['c2'][1]))
                    xt_load(n, 3)
                    weave((scores_steps(n, 1), WR['c3'][0]),
                          (projqk_steps(n, 2)
                           + oproj_steps(b, range(2, 6), act_copy=OC['c3'])
                           + projvr_steps(n, 2) + pv_steps(n, 0), WR['c3'][1]))
                    weave((scores_steps(n, 2), WR['c4'][0]),
                          (projqk_steps(n, 3)
                           + oproj_steps(b, range(6, 10), act_copy=OC['c4'])
                           + projvr_steps(n, 3) + pv_steps(n, 1), WR['c4'][1]))
                    oproj(b, range(10, 12), act_copy=OC['c5'])
                else:
                    # last batch: each score tile's pv matmuls chase its exp
                    # (both pv head-groups stay open in separate psum banks),
                    # o_proj fills the gaps, and the qc3 normalization is
                    # split by query half so the final o_proj tiles pipeline.
                    pvk = ("pvps", b, 3)
                    weave((chase_steps(b, 3, pvk), WR['l1'][0]),
                          (oproj_steps(prv, range(12, 16), act_copy=OC['l1'])
                           + oproj_steps(b, range(0, 2), act_copy=OC['l1'])
                           + pv_steps(b, 2)
                           + oproj_steps(b, range(2, 8), act_copy=OC['l1']), WR['l1'][1]))
                    if prv is not None:
                        release(prv)
                    chase_norm(b, 3, (0, 256), pvk, pop_pv=False)
                    oproj(b, range(8, 12), act_copy=OC['l2'])
                    chase_norm(b, 3, (256, 512), pvk)
                    oproj(b, range(12, 14), act_copy=OC['l3'], alt=True)
                    oproj(b, range(14, 16), act_copy=OC['l3'], alt=True)
                    release(b)

    nc.compile()
    return nc


_NC_CACHE = {}


def _get_nc(nb=B):
    if nb not in _NC_CACHE:
        _NC_CACHE[nb] = build_nc(nb)
    return _NC_CACHE[nb]


def _f8(a):
    return a.astype(F8NP)


def _host_prep(x, Wq, Wk, Wv, Wo):
    x2 = np.ascontiguousarray(x.reshape(BS, D))
    xs = (x2 * SX).astype(np.float32)
    xh = _f8(xs)
    xl = _f8(xs - xh.astype(np.float32))

    def xbt_layout(a):  # [BS, D] fp8 -> [128, 8, BS]
        return np.ascontiguousarray(a.reshape(BS, 8, 128).transpose(2, 1, 0))

    xbth = xbt_layout(xh)
    xbtl = xbt_layout(xl)

    half = 32
    inv_freq = 1.0 / (ROPE_THETA ** (np.arange(half, dtype=np.float64) / half))
    freqs = np.arange(S, dtype=np.float64)[:, None] * inv_freq[None, :]
    c_ = np.cos(freqs).astype(np.float32).T      # [32, S]
    s_ = np.sin(freqs).astype(np.float32).T
    cos1 = np.tile(c_, (4, 1))                        # [128, S]
    sins1 = np.vstack([-s_, s_, -s_, s_])             # [128, S]

    perm = np.zeros(128, dtype=np.int64)
    partner = np.zeros(128, dtype=np.int64)
    for hh in range(2):
        for j in range(64):
            perm[64 * hh + j] = 64 * hh + (2 * j if j < 32 else 2 * (j - 32) + 1)
            partner[64 * hh + j] = 64 * hh + (j + 32) % 64
    pswap = np.zeros((128, 128), dtype=np.float32)
    pswap[partner, np.arange(128)] = 1.0

    sinp1 = sins1[partner] / SCALE                    # u = ps * sinp trick
    cos1 = cos1 / SCALE                               # fold fp8 descale
    cos2 = np.ascontiguousarray(
        np.broadcast_to(cos1[:, None, :], (128, 2, S))).astype(BFNP)
    sinp2 = np.ascontiguousarray(
        np.broadcast_to(sinp1[:, None, :], (128, 2, S))).astype(BFNP)

    # maskw[p, j] = 1 if j >= p else 0, width 512 (cols >=128 all ones);
    # sliced to the exact exp'd range of each diagonal tile
    jj = np.arange(512)
    masksq = (jj[None, :] >= np.arange(128)[:, None]).astype(np.float32).astype(BFNP)

    def w_hilo(Wsl):  # [128 rows, D] (already permuted/sliced) -> hi,lo [128,8,128]
        ws = (Wsl.T * SW).astype(np.float32)          # [D, 128]
        wh = _f8(ws)
        wl = _f8(ws - wh.astype(np.float32))
        def lay(a):
            return np.ascontiguousarray(a.reshape(8, 128, 128).transpose(1, 0, 2))
        return lay(wh), lay(wl)

    in_maps = []
    for c in range(NCORES):
        sl = slice(128 * c, 128 * c + 128)
        wqh, wql = w_hilo(Wq[sl][perm])
        wkh, wkl = w_hilo(Wk[sl][perm])
        wvh, wvl = w_hilo(Wv[sl])

        def flat(a):
            return np.ascontiguousarray(a).reshape(128, -1)

        boot = np.concatenate(
            [flat(wqh), flat(wkh), flat(xbth[:, :, 0:512]), flat(xbtl[:, :, 0:512]),
             flat(wql), flat(wkl), flat(wvh), flat(wvl)], axis=1)
        in_maps.append({
            "xbth": xbth,
            "xbtl": xbtl,
            "boot": boot,
            "wot": np.ascontiguousarray(Wo[:, sl].T).astype(BFNP),
            "cos2": cos2,
            "sinp2": sinp2,
            "pswap": pswap.astype(BFNP),
            "masksq": masksq,
        })
    return in_maps


def kernel(x, Wq, Wk, Wv, Wo):
    global LAST_RESULTS
    x = np.asarray(x, dtype=np.float32)
    Wq = np.asarray(Wq, dtype=np.float32)
    Wk = np.asarray(Wk, dtype=np.float32)
    Wv = np.asarray(Wv, dtype=np.float32)
    Wo = np.asarray(Wo, dtype=np.float32)

    nc = _get_nc(B)
    in_maps = _host_prep(x, Wq, Wk, Wv, Wo)
    res = run_bass_kernel_spmd(nc, in_maps, core_ids=list(range(NCORES)),
                               trace=TRACE)
    LAST_RESULTS = res
    out = np.zeros((BS, D), dtype=np.float32)
    for c in range(NCORES):
        out += np.asarray(res.results[c]["y"]).astype(np.float32)
    return out.reshape(B, S, D)
